# revision 85
# baseline (speedup 1.0000x reference)
"""Grok1-style GQA attention (S=2048, H=6144, 48 Q heads / 8 KV heads, rope,
softcap-30, causal) as a Bass/Tile kernel sharded over 8 NeuronCores.

Sharding: tensor-parallel across heads. Core c owns Q heads 6c..6c+5 and KV
head c. Each core computes its qkv projection slice, rope, causal softcap
attention for its 6 Q heads against its single KV head, and a partial
o_proj (its 768 columns of w_o). The host sums the 8 partial outputs.

Key numerics trick: softcap bounds scores to [-30, 30], so softmax is
computed as exp(30*tanh(s/30) - 30) with a *constant* bias — no running max.

Schedule (v2): the PE is ~91% busy at the bf16 roofline, so the focus is
removing the startup/tail stalls and improving DMA line efficiency.
  A : qkv(0) as ONE hb-major pass over all 8 output blocks (8 psum banks).
      The ht chunk is amortized over 4x more PE work than pair-wise passes,
      so DMA supply (~240 GB/s) stays ahead of PE consumption. wq arrives
      as (ob, qd) chunk tiles in consumption order on 2 queues.
  B0..B2: attn(sc) interleaved with [carry epilogues + ht(sc+1) + qkv(sc+1)]
  C : attn(3) interleaved with o_proj mc-pairs x s-rows 0..11
  D : o_proj s-rows 12..15, resident mc-pairs first
DMA line sizes: ht tiles pack hb-pairs ([128,1024] = 2KB lines), wo loads
one [128,3072] tile per mc (6KB lines), out writes [128,1024] mc-pair
tiles (2KB lines). wq chunks are [128,1536] (3KB lines).

Layouts (host-prepped, all transposed so the contraction dim is on SBUF
partitions):
  ht   [4,24,128,1024] bf16 : ht[sc,hbp,p,j*512+c] = hidden[sc*512+c,(2hbp+j)*128+p]
  wq   [8,128,48,128] bf16  : wq[ob,p,hb,o] = w_qkv_core[ob*128+o, hb*128+p]
  wo   [12,128,6,512] bf16  : wo[mc,p,fb,m] = (w_o[:,core]*MULT).T[fb*128+p, mc*512+m]
  cosf/sinf [128,2048] bf16 : duplicated/sign-flipped rope tables (neox)
  triu [128,128] bf16       : triu[k,q] = 1 if q >= k else 0
"""

import sys, os
import numpy as np

sys.path.insert(0, "/opt/trn_rl_repo")

import ml_dtypes

import concourse.bass as bass
import concourse.mybir as mybir
import concourse.tile as tile
from concourse import bacc
from concourse.bass_utils import run_bass_kernel_spmd

F32 = mybir.dt.float32
BF16 = mybir.dt.bfloat16
AF = mybir.ActivationFunctionType

S = 2048
HID = 6144
D = 128
NQ = 6          # q heads per core
N_CORES = 8
SCALE = D ** -0.5
SOFTCAP = 30.0
ATTN_MULT = 0.08838834764831845
ROPE_THETA = 10000.0

N_SC = 4        # s-chunks of 512
SCW = 512
N_HB = 48       # hidden 128-blocks
N_HBP = 24      # hidden 256-blocks (pairs)
N_OB = 8        # output 128-blocks per core (6 Q | 1 K | 1 V)
N_MC = 12       # o_proj 512-col chunks
N_MCP = 6       # o_proj 1024-col chunk pairs
N_SB = 16       # s 128-blocks
N_FB = 6        # per-core o_proj feature 128-blocks (768/128)

OB_ORDER = [6, 7, 0, 1, 2, 3, 4, 5]   # K,V first so epilogues unblock attn


def build_nc():
    nc = bacc.Bacc("TRN2", target_bir_lowering=False, debug=False, num_devices=N_CORES)

    ht_d = nc.dram_tensor("ht", [N_SC, N_HBP, 128, 1024], BF16, kind="ExternalInput").ap()
    wq_d = nc.dram_tensor("wq", [N_OB, 128, N_HB, 128], BF16, kind="ExternalInput").ap()
    # phase-A layout: [hb-group of 4][ob-pair][128 p][2 ob x 4 hb x 128 o]
    wqa_d = nc.dram_tensor("wqa", [12, 4, 128, 1024], BF16, kind="ExternalInput").ap()
    wo_d = nc.dram_tensor("wo", [N_MC, 128, N_FB, SCW], BF16, kind="ExternalInput").ap()
    # single-copy rope tables: [64, S]; duplicated into [128, S] SBUF tiles
    # by two DMAs each (the sign flip folds into the rope rotate-copy)
    cosf_d = nc.dram_tensor("cosf", [64, S], BF16, kind="ExternalInput").ap()
    sinf_d = nc.dram_tensor("sinf", [64, S], BF16, kind="ExternalInput").ap()
    triu_d = nc.dram_tensor("triu", [128, 128], BF16, kind="ExternalInput").ap()
    ones_col_d = nc.dram_tensor("ones_col", [128, 1], BF16, kind="ExternalInput").ap()
    ident_d = nc.dram_tensor("ident", [128, 128], BF16, kind="ExternalInput").ap()
    negcap_d = nc.dram_tensor("negcap", [128, 1], F32, kind="ExternalInput").ap()
    out_d = nc.dram_tensor("out", [S, HID], BF16, kind="ExternalOutput").ap()

    from contextlib import ExitStack
    with tile.TileContext(nc) as tc, ExitStack() as ctx:
        const = ctx.enter_context(tc.tile_pool(name="const", bufs=1))
        ktp = ctx.enter_context(tc.tile_pool(name="ktp", bufs=4))
        vnp = ctx.enter_context(tc.tile_pool(name="vnp", bufs=4))
        aotp = ctx.enter_context(tc.tile_pool(name="aotp", bufs=24))
        qtp = ctx.enter_context(tc.tile_pool(name="qtp", bufs=11))
        vtp = ctx.enter_context(tc.tile_pool(name="vtp", bufs=1))
        htp = ctx.enter_context(tc.tile_pool(name="htp", bufs=24))
        wqp = ctx.enter_context(tc.tile_pool(name="wqp", bufs=8))
        wqa = ctx.enter_context(tc.tile_pool(name="wqa", bufs=14))
        wop = ctx.enter_context(tc.tile_pool(name="wop", bufs=5))
        ropep = ctx.enter_context(tc.tile_pool(name="ropep", bufs=1))
        tpool = ctx.enter_context(tc.tile_pool(name="tpool", bufs=2))
        ppool = ctx.enter_context(tc.tile_pool(name="ppool", bufs=3))
        rpool = ctx.enter_context(tc.tile_pool(name="rpool", bufs=1))
        bpool = ctx.enter_context(tc.tile_pool(name="bpool", bufs=1))
        outp = ctx.enter_context(tc.tile_pool(name="outp", bufs=3))
        ps_a = ctx.enter_context(tc.tile_pool(name="ps_a", bufs=2, space=bass.MemorySpace.PSUM))
        ps_s = ctx.enter_context(tc.tile_pool(name="ps_s", bufs=2, space=bass.MemorySpace.PSUM))
        ps_pv = ctx.enter_context(tc.tile_pool(name="ps_pv", bufs=2, space=bass.MemorySpace.PSUM))
        ps_o = ctx.enter_context(tc.tile_pool(name="ps_o", bufs=2, space=bass.MemorySpace.PSUM))

        cosf = const.tile([128, S], BF16, tag="cosf", name="cosf")
        sinf = const.tile([128, S], BF16, tag="sinf", name="sinf")
        triu = const.tile([128, 128], BF16, tag="triu", name="triu")
        ones_col = const.tile([128, 1], BF16, tag="ones_col", name="ones_col")
        ident = const.tile([128, 128], BF16, tag="ident", name="ident")
        negcap = const.tile([128, 1], F32, tag="negcap", name="negcap")

        # wq chunk tiles: key (sc, ob, qd) -> [128, 12*128]
        wq_pref = {}

        def load_wq_chunk(sc, ob, qd, eng):
            t = wqp.tile([128, 12 * 128], BF16, tag="wq", name="wq")
            eng.dma_start(t[:], wq_d[ob, :, qd * 12:(qd + 1) * 12])
            wq_pref[(sc, ob, qd)] = t

        def load_wq_ob(sc, ob):
            """Stage all 4 qd chunks of one ob (only used when the ring is
            empty, so the triggers never block)."""
            for qd in range(4):
                load_wq_chunk(sc, ob, qd, nc.scalar)

        # per-chunk persistent tiles, filled as the pipeline progresses
        KT = {}    # sc -> [128, 512] bf16   (k^T, d on partitions)
        VN = {}    # sc -> [128, 512] bf16   (v natural, k on partitions)
        QT = {}    # (sc, h) -> [128, 512] bf16
        AOT = {}   # (sc, h) -> [128, 512] bf16
        ht_tiles = {}   # (sc, hbp) -> [128, 1024]

        def load_ht(sc):
            """Generator: issue ht hb-pair DMAs, 4 tiles per unit, rotating
            over all three DMA queues."""
            for hbp0 in range(0, N_HBP, 4):
                for hbp in range(hbp0, hbp0 + 4):
                    t = htp.tile([128, 1024], BF16, tag="ht", name="ht")
                    nc.sync.dma_start(t[:], ht_d[sc, hbp])
                    ht_tiles[(sc, hbp)] = t
                yield

        def ht_rhs(sc, hb):
            j = hb % 2
            return ht_tiles[(sc, hb // 2)][:, j * 512:(j + 1) * 512]

        def rope_epilogue(sc, ob, ps):
            scs = slice(sc * SCW, (sc + 1) * SCW)
            rot = ropep.tile([128, SCW], F32, tag="rot", name="rot")
            # neox rotate-half with the sign folded in (sinf holds +sin twice)
            nc.scalar.activation(rot[0:64, :], ps[64:128, :], AF.Copy, scale=-1.0)
            nc.scalar.copy(rot[64:128, :], ps[0:64, :])
            t1 = ropep.tile([128, SCW], F32, tag="t1", name="t1")
            nc.vector.tensor_mul(t1[:], ps[:], cosf[:, scs])
            nc.vector.tensor_mul(rot[:], rot[:], sinf[:, scs])
            if ob < NQ:
                qt = qtp.tile([128, SCW], BF16, tag="qt", name="qt")
                QT[(sc, ob)] = qt
                nc.vector.tensor_add(qt[:], t1[:], rot[:])
            else:
                kt = ktp.tile([128, SCW], BF16, tag="kt", name="kt")
                KT[sc] = kt
                nc.vector.tensor_add(kt[:], t1[:], rot[:])

        def v_epilogue(sc, ps, tp_pool, tp_tag):
            vt = vtp.tile([128, SCW], BF16, tag="vt", name="vt")
            nc.vector.tensor_copy(vt[:], ps[:])
            vn = vnp.tile([128, SCW], BF16, tag="vn", name="vn")
            VN[sc] = vn
            for j in range(4):
                tps = tp_pool.tile([128, 128], BF16, tag=tp_tag, name="tps")
                nc.tensor.transpose(tps[:], vt[:, j * 128:(j + 1) * 128], ident[:])
                nc.vector.tensor_copy(vn[:, j * 128:(j + 1) * 128], tps[:])

        # ---- phase A: qkv(0), single hb-major pass over all 8 obs ----
        # wq arrives as [128,1024] tiles: one per (4-hb group, ob-pair), in
        # consumption order spread over three queues.  The 16-deep ring keeps
        # supply ~4 groups (~27us of PE work) ahead of consumption.
        wq_a = {}   # (g, pair) -> tile
        A_PAIR = {6: (0, 0), 7: (0, 1), 0: (1, 0), 1: (1, 1),
                  2: (2, 0), 3: (2, 1), 4: (3, 0), 5: (3, 1)}

        def wq_a_slice(ob, hb):
            pr, i = A_PAIR[ob]
            t = wq_a[(hb // 4, pr)]
            col = (i * 4 + hb % 4) * 128
            return t[:, col:col + 128]

        def load_wqa_group(g, prs=(0, 1, 2, 3)):
            engs = (nc.scalar, nc.gpsimd)
            for pr in prs:
                t = wqa.tile([128, 1024], BF16, tag="wqa", name="wqa")
                engs[(g * 4 + pr) % 2].dma_start(t[:], wqa_d[g, pr])
                wq_a[(g, pr)] = t

        def phase_a():
            """Returns the carry generator of leftover epilogues."""
            # Upfront: first 8 ht hb-pairs and 3 wq groups; the rest issue
            # group-by-group between matmuls so ring-slot waits always see
            # their slot's readers already emitted.
            ht_loader = load_ht(0)
            next(ht_loader)
            load_wqa_group(0)
            load_wqa_group(1)
            next(ht_loader)
            load_wqa_group(2)
            load_wqa_group(3)

            # 8 psum accumulators: K,V -> ps_pv; q0,q1 -> ps_s; q2,q3 -> ps_a;
            # q4,q5 -> ps_o.  Tags reuse each pool's standard tag (pools ring
            # per tag).  Freed in that tail order for B0's needs.
            pools = {6: (ps_pv, "pv"), 7: (ps_pv, "pv"), 0: (ps_s, "s"),
                     1: (ps_s, "s"), 2: (ps_a, "acc"), 3: (ps_a, "acc"),
                     4: (ps_o, "oa"), 5: (ps_o, "oa")}
            acc = {}
            for ob in OB_ORDER:
                pool, tg = pools[ob]
                acc[ob] = pool.tile([128, SCW], F32, tag=tg, name=tg)

            def mm(ob, hb):
                nc.tensor.matmul(
                    acc[ob][:], lhsT=wq_a_slice(ob, hb), rhs=ht_rhs(0, hb),
                    start=(hb == 0), stop=(hb == N_HB - 1),
                )

            ht_units_at = {8: 1, 16: 1, 24: 1, 32: 1}
            for hb in range(44):
                # group g+3 prefetch, split so each tile's ring slot already
                # has its readers emitted (14-deep ring)
                if hb >= 4 and hb % 4 == 0 and hb // 4 + 3 <= 11:
                    load_wqa_group(hb // 4 + 3, prs=(0, 1))
                if hb >= 4 and hb % 4 == 2 and (hb - 2) // 4 + 3 <= 11:
                    load_wqa_group((hb - 2) // 4 + 3, prs=(2, 3))
                for _ in range(ht_units_at.get(hb, 0)):
                    next(ht_loader)
                if hb == 20:
                    # rope tables needed at the epilogues (~80us in); sync's
                    # ht(0) traffic is nearly drained by then, while scalar/
                    # gpsimd still carry the late wqa groups
                    nc.sync.dma_start(cosf[0:64, :], cosf_d[:])
                    nc.sync.dma_start(cosf[64:128, :], cosf_d[:])
                    nc.sync.dma_start(sinf[0:64, :], sinf_d[:])
                    nc.sync.dma_start(sinf[64:128, :], sinf_d[:])
                    nc.sync.dma_start(ident[:], ident_d[:])
                    nc.sync.dma_start(triu[:], triu_d[:])
                    nc.sync.dma_start(ones_col[:], ones_col_d[:])
                    nc.sync.dma_start(negcap[:], negcap_d[:])

                for ob in OB_ORDER:
                    mm(ob, hb)
            # staggered tail: finish K,V first, then q0,q1, so their
            # epilogue chains (scalar/vector) overlap the remaining tails.
            for hb in range(44, 48):
                mm(6, hb)
                mm(7, hb)
            rope_epilogue(0, 6, acc[6])
            vt = vtp.tile([128, SCW], BF16, tag="vt", name="vt")
            nc.vector.tensor_copy(vt[:], acc[7][:])
            for hb in range(44, 48):
                mm(0, hb)
                mm(1, hb)
            rope_epilogue(0, 0, acc[0])
            rope_epilogue(0, 1, acc[1])
            for hb in range(44, 48):
                mm(2, hb)
                mm(3, hb)
            vn = vnp.tile([128, SCW], BF16, tag="vn", name="vn")
            VN[0] = vn
            for j in range(4):
                tps = ps_pv.tile([128, 128], BF16, tag="pv", name="tps")
                nc.tensor.transpose(tps[:], vt[:, j * 128:(j + 1) * 128], ident[:])
                nc.vector.tensor_copy(vn[:, j * 128:(j + 1) * 128], tps[:])
            for hb in range(44, 48):
                mm(4, hb)
                mm(5, hb)
            wq_a.clear()

            def carry_gen():
                for ob in (2, 3, 4, 5):
                    rope_epilogue(0, ob, acc[ob])
                    yield
            return carry_gen()

        def qkv_stream(sc):
            """Generator: qkv projection + rope for chunk sc (1..3). Yields at
            boundaries where attention work may be interleaved. The next ob's
            wq chunks issue one-by-one right after the emissions that free
            their ring slot, so the (engine-blocking) DMA triggers never sit
            on a long slot wait in front of latency-critical scalar work."""
            # chunk qd issues after hb-group g of the current ob, spread over
            # two queues; each reused ring slot's readers are already emitted.
            PREF = {2: (0, nc.scalar), 5: (1, nc.gpsimd),
                    8: (2, nc.scalar), 11: (3, nc.gpsimd)}
            for idx, ob in enumerate(OB_ORDER):
                if idx + 1 < N_OB:
                    nxt = (sc, OB_ORDER[idx + 1])
                elif sc < 3:
                    nxt = (sc + 1, OB_ORDER[0])
                else:
                    nxt = None
                yield
                chunks = [wq_pref.pop((sc, ob, qd)) for qd in range(4)]
                ps = ps_a.tile([128, SCW], F32, tag="acc", name="acc")
                for hb0 in range(0, N_HB, 4):
                    for hb in range(hb0, hb0 + 4):
                        w = chunks[hb // 12]
                        nc.tensor.matmul(
                            ps[:],
                            lhsT=w[:, (hb % 12) * 128:(hb % 12 + 1) * 128],
                            rhs=ht_rhs(sc, hb),
                            start=(hb == 0),
                            stop=(hb == N_HB - 1),
                        )
                    if nxt is not None and hb0 // 4 in PREF:
                        qd, eng = PREF[hb0 // 4]
                        load_wq_chunk(nxt[0], nxt[1], qd, eng)
                    yield
                if ob <= NQ:
                    rope_epilogue(sc, ob, ps)
                else:
                    v_epilogue(sc, ps, ps_a, 'acc')
                yield

        def attn_stream(qc):
            """Generator: attention for q-chunk qc, all 6 heads. Score matmuls
            run LOOK iterations ahead of PV; normalization is deferred one
            head so recip/broadcast never block the vector engine's triu."""
            nkb = 4 * qc + 4
            iters = [(h, kb) for h in range(NQ) for kb in range(nkb)]
            n = len(iters)
            LOOK = 2
            state = {}
            pv_cur = {}
            oa_cur = {}
            pend = []   # deferred (pv, bc, h) normalizations

            def issue_score(idx):
                h, kb = iters[idx]
                qs = max(qc * SCW, kb * 128)
                off = qs - qc * SCW
                w = SCW - off
                sp = ps_s.tile([128, SCW], F32, tag="s", name="s")
                nc.tensor.matmul(
                    sp[:, :w],
                    lhsT=KT[kb // 4][:, (kb % 4) * 128:(kb % 4 + 1) * 128],
                    rhs=QT[(qc, h)][:, off:SCW],
                    start=True, stop=True,
                )
                tt = tpool.tile([128, SCW], F32, tag="t", name="t")
                nc.scalar.activation(tt[:, :w], sp[:, :w], AF.Tanh,
                                     scale=SCALE / SOFTCAP)
                pt = ppool.tile([128, SCW], BF16, tag="p", name="p")
                nc.scalar.activation(pt[:, :w], tt[:, :w], AF.Exp,
                                     scale=SOFTCAP, bias=negcap[:])
                if kb >= 4 * qc:
                    nc.vector.tensor_mul(pt[:, 0:128], pt[:, 0:128], triu[:])
                state[idx] = (pt, w, off)

            def flush_norm():
                pv, bc, h = pend.pop(0)
                at = aotp.tile([128, SCW], BF16, tag="aot", name="aot")
                AOT[(qc, h)] = at
                nc.vector.tensor_mul(at[:], pv[:], bc[:])

            def issue_pv(idx):
                h, kb = iters[idx]
                pt, w, off = state.pop(idx)
                if kb == 0:
                    pv_cur[h] = ps_pv.tile([128, SCW], F32, tag="pv", name="pv")
                    oa_cur[h] = ps_o.tile([1, SCW], F32, tag="oa", name="oa")
                if kb == 1 and pend:
                    flush_norm()
                pv, oa = pv_cur[h], oa_cur[h]
                nc.tensor.matmul(
                    pv[:, off:SCW],
                    lhsT=VN[kb // 4][:, (kb % 4) * 128:(kb % 4 + 1) * 128],
                    rhs=pt[:, :w],
                    start=(kb == 0), stop=(kb == nkb - 1),
                )
                nc.tensor.matmul(
                    oa[0:1, off:SCW],
                    lhsT=ones_col[:],
                    rhs=pt[:, :w],
                    start=(kb == 0), stop=(kb == nkb - 1),
                )
                if kb == nkb - 1:
                    rr = rpool.tile([1, SCW], F32, tag="r", name="r")
                    # [1,512] single-partition DVE op: plain reciprocal costs
                    # 3.3us on the head-boundary critical chain; approx_fast
                    # (~18 bits, den is a positive normal) is 5x faster.
                    nc.vector.reciprocal_approx_fast(rr[:], oa[0:1, :])
                    bc = bpool.tile([128, SCW], F32, tag="bc", name="bc")
                    nc.gpsimd.partition_broadcast(bc[:], rr[:])
                    pend.append((pv, bc, h))

            for j in range(min(LOOK, n)):
                issue_score(j)
            for i in range(n):
                if i + LOOK < n:
                    issue_score(i + LOOK)
                yield
                issue_pv(i)
            while pend:
                flush_norm()

        # ---- o_proj: mc-pair granularity, [128,1024] out tiles ----
        wo_tiles = {}

        def load_wo(mc, eng=None):
            wos = wop.tile([128, N_FB * SCW], BF16, tag="wo", name="wo")
            (eng or nc.sync).dma_start(wos[:], wo_d[mc])
            wo_tiles[mc] = wos

        def oproj_row(sb, mcp, wr_eng=None, split_wr=False):
            """One [128,1024] output tile: s-block sb x mc pair (2mcp, 2mcp+1)."""
            sc, j = sb // 4, sb % 4
            ot = outp.tile([128, 1024], BF16, tag="out", name="out")
            for half in range(2):
                mc = 2 * mcp + half
                wos = wo_tiles[mc]
                op = ps_a.tile([128, SCW], F32, tag="acc", name="acc")
                for fb in range(N_FB):
                    nc.tensor.matmul(
                        op[:],
                        lhsT=AOT[(sc, fb)][:, j * 128:(j + 1) * 128],
                        rhs=wos[:, fb * SCW:(fb + 1) * SCW],
                        start=(fb == 0), stop=(fb == N_FB - 1),
                    )
                if half == 0:
                    nc.vector.tensor_copy(ot[:, 0:512], op[:])
                    if split_wr:  # drain the tail write in parallel halves
                        nc.scalar.dma_start(
                            out_d[sb * 128:(sb + 1) * 128,
                                  mcp * 1024:mcp * 1024 + 512], ot[:, 0:512])
                else:
                    nc.scalar.copy(ot[:, 512:1024], op[:])
            if split_wr:
                nc.sync.dma_start(
                    out_d[sb * 128:(sb + 1) * 128,
                          mcp * 1024 + 512:(mcp + 1) * 1024], ot[:, 512:1024])
            else:
                (wr_eng or nc.sync).dma_start(
                    out_d[sb * 128:(sb + 1) * 128, mcp * 1024:(mcp + 1) * 1024], ot[:])

        def oproj_c_stream():
            """Phase C: mcp-outer over s-rows 0..11. wo(0,1) preloaded in B2;
            pairs load just in time on gpsimd (sync carries the out writes);
            ring keeps the last 6 mc for phase D."""
            for mcp in range(N_MCP):
                if mcp + 1 < N_MCP:
                    load_wo(2 * mcp + 2, nc.gpsimd)
                    load_wo(2 * mcp + 3, nc.gpsimd)
                yield
                for sb in range(12):
                    oproj_row(sb, mcp)
                    yield

        def oproj_d():
            """Phase D: s-rows 12..15. mc 7..11 still in the wo ring (bufs=5);
            mc 0..5 reload staggered behind the resident mcps' rows. Writes
            alternate scalar/sync; wo pairs split gpsimd/sync."""
            def rows(mcp, last=False):
                for sb in range(12, 16):
                    oproj_row(sb, mcp, wr_eng=nc.scalar if sb % 2 else nc.sync,
                              split_wr=(last and sb == 15))
            rows(4)
            load_wo(6, nc.gpsimd)
            load_wo(7, nc.sync)
            rows(5)
            load_wo(0, nc.gpsimd)
            load_wo(1, nc.sync)
            rows(3)
            load_wo(2, nc.gpsimd)
            load_wo(3, nc.sync)
            rows(0)
            load_wo(4, nc.gpsimd)
            load_wo(5, nc.sync)
            rows(1)
            rows(2, last=True)

        def chain(*gens):
            for g in gens:
                yield from g

        def interleave(primary, filler, n_primary, n_filler, reserve=0,
                       ratio=None, drain=True):
            """Advance primary; between slots advance filler so both streams
            finish together (adaptive, or fixed `ratio`). Keep `reserve`
            filler units unexecuted; drain (or hand back) the remainder."""
            rem_p, rem_f = n_primary, n_filler
            acc = 0.0
            f_done = False
            for _ in primary:
                rem_p -= 1
                if not f_done:
                    acc += ratio if ratio is not None else rem_f / max(rem_p, 1)
                    while acc >= 1.0 and not f_done and rem_f > reserve:
                        try:
                            next(filler)
                            rem_f -= 1
                        except StopIteration:
                            f_done = True
                        acc -= 1.0
            if drain and not f_done:
                for _ in filler:
                    pass
            return filler if not f_done else None

        # ---- schedule ----
        carry = phase_a()

        def wo_preload():
            load_wo(0)
            load_wo(1)
            yield

        def wq_stage_b0():
            # qkv(1)'s first ob, staged at B0 start once scalar is free of
            # phase-A wqa traffic (keeps 1.6MB out of the phase-A window)
            load_wq_ob(1, OB_ORDER[0])
            yield

        # B0..B2: attn(sc) ⋈ [carry + ht(sc+1) (+ wo preload) + qkv(sc+1)]
        for sc in range(3):
            primary = attn_stream(sc)
            parts = [carry] if carry is not None else []
            if sc == 0:
                parts += [wq_stage_b0()]
            parts += [load_ht(sc + 1)]
            if sc == 2:  # stage wo(0..1) early for phase C
                parts += [wo_preload()]
            parts += [qkv_stream(sc + 1)]
            n_carry = 4 if sc == 0 else 0
            filler = chain(*parts)
            n_primary = NQ * (4 * sc + 4)
            n_filler = n_carry + 6 + 14 * N_OB + (1 if sc == 2 else 0)
            carry = interleave(primary, filler, n_primary, n_filler,
                               reserve=14)
        if carry is not None:
            for _ in carry:
                pass

        # C: attn(3) ⋈ o_proj rows 0..11 (adaptive ratio: 78 filler units
        # must last all 96 primary slots or the attn tail runs PE-starved)
        primary = attn_stream(3)
        filler = oproj_c_stream()
        interleave(primary, filler, NQ * 16, N_MCP * 13)

        # D: o_proj rows 12..15
        oproj_d()

    nc.compile()
    return nc


def prep_inputs(positions, hidden_states, w_qkv, w_o):
    """Host-side shard + relayout. Returns per-core input maps."""
    bf = ml_dtypes.bfloat16
    pos = np.asarray(positions).astype(np.float32)
    hidden = np.ascontiguousarray(np.asarray(hidden_states, dtype=np.float32))
    w_qkv = np.asarray(w_qkv, dtype=np.float32)
    w_o = np.asarray(w_o, dtype=np.float32)

    # rope tables (neox): freqs [S, 64]
    inv_freq = 1.0 / (ROPE_THETA ** (np.arange(0, D, 2, dtype=np.float32) / D))
    freqs = pos[:, None] * inv_freq[None, :]
    cosf = np.cos(freqs).T.astype(np.float32).astype(bf)   # [64, S]
    sinf = np.sin(freqs).T.astype(np.float32).astype(bf)   # [64, S]; sign
    # flip is folded into the rope rotate-copy (scale=-1)

    triu = np.triu(np.ones((128, 128), np.float32)).astype(bf)  # [k, q]: q >= k
    ones_col = np.ones((128, 1), np.float32).astype(bf)
    ident = np.eye(128, dtype=np.float32).astype(bf)

    # ht[sc, hbp, p, j*512+c] = hidden[sc*512+c, (2*hbp+j)*128+p]
    ht = np.ascontiguousarray(
        hidden.reshape(N_SC, SCW, N_HBP, 2, 128).transpose(0, 2, 4, 3, 1)
    ).reshape(N_SC, N_HBP, 128, 1024).astype(bf)

    in_maps = []
    for c in range(N_CORES):
        q_rows = w_qkv[c * NQ * D:(c + 1) * NQ * D]          # [768, 6144]
        k_rows = w_qkv[HID + c * D:HID + (c + 1) * D]        # [128, 6144]
        v_rows = w_qkv[HID + 8 * D + c * D:HID + 8 * D + (c + 1) * D]
        wq_c = np.concatenate([q_rows, k_rows, v_rows], axis=0)  # [1024, 6144]
        # wq[ob, p, hb, o] = wq_c[ob*128+o, hb*128+p]
        wq_arr = np.ascontiguousarray(
            wq_c.reshape(N_OB, 128, N_HB, 128).transpose(0, 3, 2, 1)).astype(bf)
        # wqa[g, pr, p, (i*4+m)*128+o] = wq_c[pair[pr][i]*128+o, (4g+m)*128+p]
        wq_blk = wq_c.reshape(N_OB, 128, 12, 4, 128)  # [ob, o, g, m, p]
        wqa_arr = np.empty((12, 4, 128, 1024), np.float32)
        for pr, obs in enumerate(((6, 7), (0, 1), (2, 3), (4, 5))):
            blk = wq_blk[list(obs)]                   # [i, o, g, m, p]
            wqa_arr[:, pr] = blk.transpose(2, 4, 0, 3, 1).reshape(12, 128, 1024)
        wqa_arr = np.ascontiguousarray(wqa_arr).astype(bf)
        wo_c = (w_o[:, c * NQ * D:(c + 1) * NQ * D] * ATTN_MULT).T  # [768, 6144]
        # wo[mc, p, fb, m] = wo_c[fb*128+p, mc*512+m]
        wo_arr = np.ascontiguousarray(
            wo_c.reshape(N_FB, 128, N_MC, SCW).transpose(2, 1, 0, 3)).astype(bf)
        in_maps.append({
            "ht": ht, "wq": wq_arr, "wqa": wqa_arr, "wo": wo_arr,
            "cosf": cosf, "sinf": sinf, "triu": triu,
            "ones_col": ones_col, "ident": ident,
            "negcap": np.full((128, 1), -SOFTCAP, np.float32),
        })
    return in_maps


_NC_CACHE = None


def _get_nc():
    global _NC_CACHE
    if _NC_CACHE is None:
        _NC_CACHE = build_nc()
    return _NC_CACHE


def kernel(positions, hidden_states, w_qkv, w_o, _trace=False, _trace_kwargs=None):
    nc = _get_nc()
    in_maps = prep_inputs(positions, hidden_states, w_qkv, w_o)
    res = run_bass_kernel_spmd(nc, in_maps, list(range(N_CORES)),
                               trace=_trace, **(_trace_kwargs or {}))
    out = np.zeros((S, HID), np.float32)
    for c in range(N_CORES):
        out += np.asarray(res.results[c]["out"]).astype(np.float32)
    out = out.astype(np.asarray(hidden_states).dtype)
    kernel.last_results = res
    return out


# revision 91
# speedup vs baseline: 1.0166x; 1.0166x over previous
"""Grok1-style GQA attention (S=2048, H=6144, 48 Q heads / 8 KV heads, rope,
softcap-30, causal) as a Bass/Tile kernel sharded over 8 NeuronCores.

Sharding: tensor-parallel across heads. Core c owns Q heads 6c..6c+5 and KV
head c. Each core computes its qkv projection slice, rope, causal softcap
attention for its 6 Q heads against its single KV head, and a partial
o_proj (its 768 columns of w_o). The host sums the 8 partial outputs.

Key numerics trick: softcap bounds scores to [-30, 30], so softmax is
computed as exp(30*tanh(s/30) - 30) with a *constant* bias — no running max.

Schedule (v2): the PE is ~91% busy at the bf16 roofline, so the focus is
removing the startup/tail stalls and improving DMA line efficiency.
  A : qkv(0) as ONE hb-major pass over all 8 output blocks (8 psum banks).
      The ht chunk is amortized over 4x more PE work than pair-wise passes,
      so DMA supply (~240 GB/s) stays ahead of PE consumption. wq arrives
      as (ob, qd) chunk tiles in consumption order on 2 queues.
  B0..B2: attn(sc) interleaved with [carry epilogues + ht(sc+1) + qkv(sc+1)]
  C : attn(3) interleaved with o_proj mc-pairs x s-rows 0..11
  D : o_proj s-rows 12..15, resident mc-pairs first
DMA line sizes: ht tiles pack hb-pairs ([128,1024] = 2KB lines), wo loads
one [128,3072] tile per mc (6KB lines), out writes [128,1024] mc-pair
tiles (2KB lines). wq chunks are [128,1536] (3KB lines).

Layouts (host-prepped, all transposed so the contraction dim is on SBUF
partitions):
  ht   [4,24,128,1024] bf16 : ht[sc,hbp,p,j*512+c] = hidden[sc*512+c,(2hbp+j)*128+p]
  wq   [8,128,48,128] bf16  : wq[ob,p,hb,o] = w_qkv_core[ob*128+o, hb*128+p]
  wo   [12,128,6,512] bf16  : wo[mc,p,fb,m] = (w_o[:,core]*MULT).T[fb*128+p, mc*512+m]
  cosf/sinf [128,2048] bf16 : duplicated/sign-flipped rope tables (neox)
  triu [128,128] bf16       : triu[k,q] = 1 if q >= k else 0
"""

import sys, os
import numpy as np

sys.path.insert(0, "/opt/trn_rl_repo")

import ml_dtypes

import concourse.bass as bass
import concourse.mybir as mybir
import concourse.tile as tile
from concourse import bacc
from concourse.bass_utils import run_bass_kernel_spmd

F32 = mybir.dt.float32
BF16 = mybir.dt.bfloat16
AF = mybir.ActivationFunctionType

S = 2048
HID = 6144
D = 128
NQ = 6          # q heads per core
N_CORES = 8
SCALE = D ** -0.5
SOFTCAP = 30.0
ATTN_MULT = 0.08838834764831845
ROPE_THETA = 10000.0

N_SC = 4        # s-chunks of 512
SCW = 512
N_HB = 48       # hidden 128-blocks
N_HBP = 24      # hidden 256-blocks (pairs)
N_OB = 8        # output 128-blocks per core (6 Q | 1 K | 1 V)
N_MC = 12       # o_proj 512-col chunks
N_MCP = 6       # o_proj 1024-col chunk pairs
N_SB = 16       # s 128-blocks
N_FB = 6        # per-core o_proj feature 128-blocks (768/128)

OB_ORDER = [6, 7, 0, 1, 2, 3, 4, 5]   # K,V first so epilogues unblock attn


def build_nc():
    nc = bacc.Bacc("TRN2", target_bir_lowering=False, debug=False, num_devices=N_CORES)

    ht_d = nc.dram_tensor("ht", [N_SC, N_HBP, 128, 1024], BF16, kind="ExternalInput").ap()
    wq_d = nc.dram_tensor("wq", [N_OB, 128, N_HB, 128], BF16, kind="ExternalInput").ap()
    # phase-A layout: [hb-group of 4][ob-pair][128 p][2 ob x 4 hb x 128 o]
    wqa_d = nc.dram_tensor("wqa", [12, 4, 128, 1024], BF16, kind="ExternalInput").ap()
    wo_d = nc.dram_tensor("wo", [N_MC, 128, N_FB, SCW], BF16, kind="ExternalInput").ap()
    cosf_d = nc.dram_tensor("cosf", [128, S], BF16, kind="ExternalInput").ap()
    sinf_d = nc.dram_tensor("sinf", [128, S], BF16, kind="ExternalInput").ap()
    triu_d = nc.dram_tensor("triu", [128, 128], BF16, kind="ExternalInput").ap()
    ones_col_d = nc.dram_tensor("ones_col", [128, 1], BF16, kind="ExternalInput").ap()
    ident_d = nc.dram_tensor("ident", [128, 128], BF16, kind="ExternalInput").ap()
    negcap_d = nc.dram_tensor("negcap", [128, 1], F32, kind="ExternalInput").ap()
    out_d = nc.dram_tensor("out", [S, HID], BF16, kind="ExternalOutput").ap()

    from contextlib import ExitStack
    with tile.TileContext(nc) as tc, ExitStack() as ctx:
        const = ctx.enter_context(tc.tile_pool(name="const", bufs=1))
        ktp = ctx.enter_context(tc.tile_pool(name="ktp", bufs=4))
        vnp = ctx.enter_context(tc.tile_pool(name="vnp", bufs=4))
        aotp = ctx.enter_context(tc.tile_pool(name="aotp", bufs=24))
        qtp = ctx.enter_context(tc.tile_pool(name="qtp", bufs=11))
        vtp = ctx.enter_context(tc.tile_pool(name="vtp", bufs=1))
        htp = ctx.enter_context(tc.tile_pool(name="htp", bufs=24))
        wqp = ctx.enter_context(tc.tile_pool(name="wqp", bufs=8))
        wqa = ctx.enter_context(tc.tile_pool(name="wqa", bufs=14))
        wop = ctx.enter_context(tc.tile_pool(name="wop", bufs=5))
        ropep = ctx.enter_context(tc.tile_pool(name="ropep", bufs=1))
        tpool = ctx.enter_context(tc.tile_pool(name="tpool", bufs=2))
        ppool = ctx.enter_context(tc.tile_pool(name="ppool", bufs=3))
        rpool = ctx.enter_context(tc.tile_pool(name="rpool", bufs=1))
        bpool = ctx.enter_context(tc.tile_pool(name="bpool", bufs=1))
        outp = ctx.enter_context(tc.tile_pool(name="outp", bufs=3))
        ps_a = ctx.enter_context(tc.tile_pool(name="ps_a", bufs=2, space=bass.MemorySpace.PSUM))
        ps_s = ctx.enter_context(tc.tile_pool(name="ps_s", bufs=2, space=bass.MemorySpace.PSUM))
        ps_pv = ctx.enter_context(tc.tile_pool(name="ps_pv", bufs=2, space=bass.MemorySpace.PSUM))
        ps_o = ctx.enter_context(tc.tile_pool(name="ps_o", bufs=2, space=bass.MemorySpace.PSUM))

        cosf = const.tile([128, S], BF16, tag="cosf", name="cosf")
        sinf = const.tile([128, S], BF16, tag="sinf", name="sinf")
        triu = const.tile([128, 128], BF16, tag="triu", name="triu")
        ones_col = const.tile([128, 1], BF16, tag="ones_col", name="ones_col")
        ident = const.tile([128, 128], BF16, tag="ident", name="ident")
        negcap = const.tile([128, 1], F32, tag="negcap", name="negcap")

        # wq chunk tiles: key (sc, ob, qd) -> [128, 12*128]
        wq_pref = {}

        def load_wq_chunk(sc, ob, qd, eng):
            t = wqp.tile([128, 12 * 128], BF16, tag="wq", name="wq")
            eng.dma_start(t[:], wq_d[ob, :, qd * 12:(qd + 1) * 12])
            wq_pref[(sc, ob, qd)] = t

        def load_wq_ob(sc, ob):
            """Stage all 4 qd chunks of one ob (only used when the ring is
            empty, so the triggers never block)."""
            for qd in range(4):
                load_wq_chunk(sc, ob, qd, nc.scalar)

        # per-chunk persistent tiles, filled as the pipeline progresses
        KT = {}    # sc -> [128, 512] bf16   (k^T, d on partitions)
        VN = {}    # sc -> [128, 512] bf16   (v natural, k on partitions)
        QT = {}    # (sc, h) -> [128, 512] bf16
        AOT = {}   # (sc, h) -> [128, 512] bf16
        ht_tiles = {}   # (sc, hbp) -> [128, 1024]

        def load_ht(sc):
            """Generator: issue ht hb-pair DMAs, 4 tiles per unit, rotating
            over all three DMA queues."""
            for hbp0 in range(0, N_HBP, 4):
                for hbp in range(hbp0, hbp0 + 4):
                    t = htp.tile([128, 1024], BF16, tag="ht", name="ht")
                    nc.sync.dma_start(t[:], ht_d[sc, hbp])
                    ht_tiles[(sc, hbp)] = t
                yield

        def ht_rhs(sc, hb):
            j = hb % 2
            return ht_tiles[(sc, hb // 2)][:, j * 512:(j + 1) * 512]

        def rope_epilogue(sc, ob, ps):
            scs = slice(sc * SCW, (sc + 1) * SCW)
            rot = ropep.tile([128, SCW], F32, tag="rot", name="rot")
            nc.scalar.copy(rot[0:64, :], ps[64:128, :])
            nc.scalar.copy(rot[64:128, :], ps[0:64, :])
            t1 = ropep.tile([128, SCW], F32, tag="t1", name="t1")
            nc.vector.tensor_mul(t1[:], ps[:], cosf[:, scs])
            nc.vector.tensor_mul(rot[:], rot[:], sinf[:, scs])
            if ob < NQ:
                qt = qtp.tile([128, SCW], BF16, tag="qt", name="qt")
                QT[(sc, ob)] = qt
                nc.vector.tensor_add(qt[:], t1[:], rot[:])
            else:
                kt = ktp.tile([128, SCW], BF16, tag="kt", name="kt")
                KT[sc] = kt
                nc.vector.tensor_add(kt[:], t1[:], rot[:])

        def v_epilogue(sc, ps, tp_pool, tp_tag):
            vt = vtp.tile([128, SCW], BF16, tag="vt", name="vt")
            nc.vector.tensor_copy(vt[:], ps[:])
            vn = vnp.tile([128, SCW], BF16, tag="vn", name="vn")
            VN[sc] = vn
            for j in range(4):
                tps = tp_pool.tile([128, 128], BF16, tag=tp_tag, name="tps")
                nc.tensor.transpose(tps[:], vt[:, j * 128:(j + 1) * 128], ident[:])
                nc.vector.tensor_copy(vn[:, j * 128:(j + 1) * 128], tps[:])

        # ---- phase A: qkv(0), single hb-major pass over all 8 obs ----
        # wq arrives as [128,1024] tiles: one per (4-hb group, ob-pair), in
        # consumption order spread over three queues.  The 16-deep ring keeps
        # supply ~4 groups (~27us of PE work) ahead of consumption.
        wq_a = {}   # (g, pair) -> tile
        A_PAIR = {6: (0, 0), 7: (0, 1), 0: (1, 0), 1: (1, 1),
                  2: (2, 0), 3: (2, 1), 4: (3, 0), 5: (3, 1)}

        def wq_a_slice(ob, hb):
            pr, i = A_PAIR[ob]
            t = wq_a[(hb // 4, pr)]
            col = (i * 4 + hb % 4) * 128
            return t[:, col:col + 128]

        def load_wqa_group(g, prs=(0, 1, 2, 3)):
            engs = (nc.scalar, nc.gpsimd)
            for pr in prs:
                t = wqa.tile([128, 1024], BF16, tag="wqa", name="wqa")
                engs[(g * 4 + pr) % 2].dma_start(t[:], wqa_d[g, pr])
                wq_a[(g, pr)] = t

        def phase_a():
            """Returns the carry generator of leftover epilogues."""
            # Upfront: first 8 ht hb-pairs and 3 wq groups; the rest issue
            # group-by-group between matmuls so ring-slot waits always see
            # their slot's readers already emitted.
            ht_loader = load_ht(0)
            next(ht_loader)
            load_wqa_group(0)
            load_wqa_group(1)
            next(ht_loader)
            load_wqa_group(2)
            load_wqa_group(3)

            # 8 psum accumulators: K,V -> ps_pv; q0,q1 -> ps_s; q2,q3 -> ps_a;
            # q4,q5 -> ps_o.  Tags reuse each pool's standard tag (pools ring
            # per tag).  Freed in that tail order for B0's needs.
            pools = {6: (ps_pv, "pv"), 7: (ps_pv, "pv"), 0: (ps_s, "s"),
                     1: (ps_s, "s"), 2: (ps_a, "acc"), 3: (ps_a, "acc"),
                     4: (ps_o, "oa"), 5: (ps_o, "oa")}
            acc = {}
            for ob in OB_ORDER:
                pool, tg = pools[ob]
                acc[ob] = pool.tile([128, SCW], F32, tag=tg, name=tg)

            def mm(ob, hb):
                nc.tensor.matmul(
                    acc[ob][:], lhsT=wq_a_slice(ob, hb), rhs=ht_rhs(0, hb),
                    start=(hb == 0), stop=(hb == N_HB - 1),
                )

            ht_units_at = {8: 1, 16: 1, 24: 1, 32: 1}
            for hb in range(44):
                # group g+3 prefetch, split so each tile's ring slot already
                # has its readers emitted (14-deep ring)
                if hb >= 4 and hb % 4 == 0 and hb // 4 + 3 <= 11:
                    load_wqa_group(hb // 4 + 3, prs=(0, 1))
                if hb >= 4 and hb % 4 == 2 and (hb - 2) // 4 + 3 <= 11:
                    load_wqa_group((hb - 2) // 4 + 3, prs=(2, 3))
                for _ in range(ht_units_at.get(hb, 0)):
                    next(ht_loader)
                if hb == 20:
                    # rope tables needed at the epilogues (~80us in); sync's
                    # ht(0) traffic is nearly drained by then, while scalar/
                    # gpsimd still carry the late wqa groups
                    nc.sync.dma_start(cosf[:], cosf_d[:])
                    nc.sync.dma_start(sinf[:], sinf_d[:])
                    nc.sync.dma_start(ident[:], ident_d[:])
                    nc.sync.dma_start(triu[:], triu_d[:])
                    nc.sync.dma_start(ones_col[:], ones_col_d[:])
                    nc.sync.dma_start(negcap[:], negcap_d[:])

                for ob in OB_ORDER:
                    mm(ob, hb)
            # staggered tail: finish K,V first, then q0,q1, so their
            # epilogue chains (scalar/vector) overlap the remaining tails.
            for hb in range(44, 48):
                mm(6, hb)
                mm(7, hb)
            rope_epilogue(0, 6, acc[6])
            vt = vtp.tile([128, SCW], BF16, tag="vt", name="vt")
            nc.vector.tensor_copy(vt[:], acc[7][:])
            for hb in range(44, 48):
                mm(0, hb)
                mm(1, hb)
            rope_epilogue(0, 0, acc[0])
            rope_epilogue(0, 1, acc[1])
            for hb in range(44, 48):
                mm(2, hb)
                mm(3, hb)
            vn = vnp.tile([128, SCW], BF16, tag="vn", name="vn")
            VN[0] = vn
            for j in range(4):
                tps = ps_pv.tile([128, 128], BF16, tag="pv", name="tps")
                nc.tensor.transpose(tps[:], vt[:, j * 128:(j + 1) * 128], ident[:])
                nc.vector.tensor_copy(vn[:, j * 128:(j + 1) * 128], tps[:])
            for hb in range(44, 48):
                mm(4, hb)
                mm(5, hb)
            wq_a.clear()
            load_wq_ob(1, OB_ORDER[0])  # stage qkv(1)'s first ob for B0

            def carry_gen():
                for ob in (2, 3, 4, 5):
                    rope_epilogue(0, ob, acc[ob])
                    yield
            return carry_gen()

        def qkv_stream(sc):
            """Generator: qkv projection + rope for chunk sc (1..3). Yields at
            boundaries where attention work may be interleaved. The next ob's
            wq chunks issue one-by-one right after the emissions that free
            their ring slot, so the (engine-blocking) DMA triggers never sit
            on a long slot wait in front of latency-critical scalar work."""
            # chunk qd issues after hb-group g of the current ob, spread over
            # two queues; each reused ring slot's readers are already emitted.
            PREF = {2: (0, nc.scalar), 5: (1, nc.gpsimd),
                    8: (2, nc.scalar), 11: (3, nc.gpsimd)}
            for idx, ob in enumerate(OB_ORDER):
                if idx + 1 < N_OB:
                    nxt = (sc, OB_ORDER[idx + 1])
                elif sc < 3:
                    nxt = (sc + 1, OB_ORDER[0])
                else:
                    nxt = None
                yield
                chunks = [wq_pref.pop((sc, ob, qd)) for qd in range(4)]
                ps = ps_a.tile([128, SCW], F32, tag="acc", name="acc")
                for hb0 in range(0, N_HB, 4):
                    for hb in range(hb0, hb0 + 4):
                        w = chunks[hb // 12]
                        nc.tensor.matmul(
                            ps[:],
                            lhsT=w[:, (hb % 12) * 128:(hb % 12 + 1) * 128],
                            rhs=ht_rhs(sc, hb),
                            start=(hb == 0),
                            stop=(hb == N_HB - 1),
                        )
                    if nxt is not None and hb0 // 4 in PREF:
                        qd, eng = PREF[hb0 // 4]
                        load_wq_chunk(nxt[0], nxt[1], qd, eng)
                    yield
                if ob <= NQ:
                    rope_epilogue(sc, ob, ps)
                else:
                    v_epilogue(sc, ps, ps_a, 'acc')
                yield

        def attn_stream(qc):
            """Generator: attention for q-chunk qc, all 6 heads. Score matmuls
            run LOOK iterations ahead of PV; normalization is deferred one
            head so recip/broadcast never block the vector engine's triu."""
            nkb = 4 * qc + 4
            iters = [(h, kb) for h in range(NQ) for kb in range(nkb)]
            n = len(iters)
            LOOK = 2
            state = {}
            pv_cur = {}
            oa_cur = {}
            pend = []   # deferred (pv, bc, h) normalizations

            def issue_score(idx):
                h, kb = iters[idx]
                qs = max(qc * SCW, kb * 128)
                off = qs - qc * SCW
                w = SCW - off
                sp = ps_s.tile([128, SCW], F32, tag="s", name="s")
                nc.tensor.matmul(
                    sp[:, :w],
                    lhsT=KT[kb // 4][:, (kb % 4) * 128:(kb % 4 + 1) * 128],
                    rhs=QT[(qc, h)][:, off:SCW],
                    start=True, stop=True,
                )
                tt = tpool.tile([128, SCW], F32, tag="t", name="t")
                nc.scalar.activation(tt[:, :w], sp[:, :w], AF.Tanh,
                                     scale=SCALE / SOFTCAP)
                pt = ppool.tile([128, SCW], BF16, tag="p", name="p")
                nc.scalar.activation(pt[:, :w], tt[:, :w], AF.Exp,
                                     scale=SOFTCAP, bias=negcap[:])
                if kb >= 4 * qc:
                    nc.vector.tensor_mul(pt[:, 0:128], pt[:, 0:128], triu[:])
                state[idx] = (pt, w, off)

            def flush_norm():
                pv, bc, h = pend.pop(0)
                at = aotp.tile([128, SCW], BF16, tag="aot", name="aot")
                AOT[(qc, h)] = at
                nc.vector.tensor_mul(at[:], pv[:], bc[:])

            def issue_pv(idx):
                h, kb = iters[idx]
                pt, w, off = state.pop(idx)
                if kb == 0:
                    pv_cur[h] = ps_pv.tile([128, SCW], F32, tag="pv", name="pv")
                    oa_cur[h] = ps_o.tile([1, SCW], F32, tag="oa", name="oa")
                if kb == 1 and pend:
                    flush_norm()
                pv, oa = pv_cur[h], oa_cur[h]
                nc.tensor.matmul(
                    pv[:, off:SCW],
                    lhsT=VN[kb // 4][:, (kb % 4) * 128:(kb % 4 + 1) * 128],
                    rhs=pt[:, :w],
                    start=(kb == 0), stop=(kb == nkb - 1),
                )
                nc.tensor.matmul(
                    oa[0:1, off:SCW],
                    lhsT=ones_col[:],
                    rhs=pt[:, :w],
                    start=(kb == 0), stop=(kb == nkb - 1),
                )
                if kb == nkb - 1:
                    rr = rpool.tile([1, SCW], F32, tag="r", name="r")
                    # [1,512] single-partition DVE op: plain reciprocal costs
                    # 3.3us on the head-boundary critical chain; approx_fast
                    # (~18 bits, den is a positive normal) is 5x faster.
                    nc.vector.reciprocal_approx_fast(rr[:], oa[0:1, :])
                    bc = bpool.tile([128, SCW], F32, tag="bc", name="bc")
                    nc.gpsimd.partition_broadcast(bc[:], rr[:])
                    pend.append((pv, bc, h))

            for j in range(min(LOOK, n)):
                issue_score(j)
            for i in range(n):
                if i + LOOK < n:
                    issue_score(i + LOOK)
                yield
                issue_pv(i)
            while pend:
                flush_norm()

        # ---- o_proj: mc-pair granularity, [128,1024] out tiles ----
        wo_tiles = {}

        def load_wo(mc, eng=None):
            wos = wop.tile([128, N_FB * SCW], BF16, tag="wo", name="wo")
            (eng or nc.sync).dma_start(wos[:], wo_d[mc])
            wo_tiles[mc] = wos

        def oproj_row(sb, mcp, wr_eng=None, split_wr=False):
            """One [128,1024] output tile: s-block sb x mc pair (2mcp, 2mcp+1)."""
            sc, j = sb // 4, sb % 4
            ot = outp.tile([128, 1024], BF16, tag="out", name="out")
            for half in range(2):
                mc = 2 * mcp + half
                wos = wo_tiles[mc]
                op = ps_a.tile([128, SCW], F32, tag="acc", name="acc")
                for fb in range(N_FB):
                    nc.tensor.matmul(
                        op[:],
                        lhsT=AOT[(sc, fb)][:, j * 128:(j + 1) * 128],
                        rhs=wos[:, fb * SCW:(fb + 1) * SCW],
                        start=(fb == 0), stop=(fb == N_FB - 1),
                    )
                if half == 0:
                    nc.vector.tensor_copy(ot[:, 0:512], op[:])
                    if split_wr:  # drain the tail write in parallel halves
                        nc.scalar.dma_start(
                            out_d[sb * 128:(sb + 1) * 128,
                                  mcp * 1024:mcp * 1024 + 512], ot[:, 0:512])
                else:
                    nc.scalar.copy(ot[:, 512:1024], op[:])
            if split_wr:
                nc.sync.dma_start(
                    out_d[sb * 128:(sb + 1) * 128,
                          mcp * 1024 + 512:(mcp + 1) * 1024], ot[:, 512:1024])
            else:
                (wr_eng or nc.sync).dma_start(
                    out_d[sb * 128:(sb + 1) * 128, mcp * 1024:(mcp + 1) * 1024], ot[:])

        def oproj_c_stream():
            """Phase C: mcp-outer over s-rows 0..11. wo(0,1) preloaded in B2;
            pairs load just in time on gpsimd (sync carries the out writes);
            ring keeps the last 6 mc for phase D."""
            for mcp in range(N_MCP):
                if mcp + 1 < N_MCP:
                    load_wo(2 * mcp + 2, nc.gpsimd)
                    load_wo(2 * mcp + 3, nc.gpsimd)
                yield
                for sb in range(12):
                    oproj_row(sb, mcp)
                    yield

        def oproj_d():
            """Phase D: s-rows 12..15. mc 7..11 still in the wo ring (bufs=5);
            mc 0..5 reload staggered behind the resident mcps' rows. Writes
            alternate scalar/sync; wo pairs split gpsimd/sync."""
            def rows(mcp, last=False):
                for sb in range(12, 16):
                    oproj_row(sb, mcp, wr_eng=nc.scalar if sb % 2 else nc.sync,
                              split_wr=(last and sb == 15))
            rows(4)
            load_wo(6, nc.gpsimd)
            load_wo(7, nc.sync)
            rows(5)
            load_wo(0, nc.gpsimd)
            load_wo(1, nc.sync)
            rows(3)
            load_wo(2, nc.gpsimd)
            load_wo(3, nc.sync)
            rows(0)
            load_wo(4, nc.gpsimd)
            load_wo(5, nc.sync)
            rows(1)
            rows(2, last=True)

        def chain(*gens):
            for g in gens:
                yield from g

        def interleave(primary, filler, n_primary, n_filler, reserve=0,
                       ratio=None, drain=True):
            """Advance primary; between slots advance filler so both streams
            finish together (adaptive, or fixed `ratio`). Keep `reserve`
            filler units unexecuted; drain (or hand back) the remainder."""
            rem_p, rem_f = n_primary, n_filler
            acc = 0.0
            f_done = False
            for _ in primary:
                rem_p -= 1
                if not f_done:
                    acc += ratio if ratio is not None else rem_f / max(rem_p, 1)
                    while acc >= 1.0 and not f_done and rem_f > reserve:
                        try:
                            next(filler)
                            rem_f -= 1
                        except StopIteration:
                            f_done = True
                        acc -= 1.0
            if drain and not f_done:
                for _ in filler:
                    pass
            return filler if not f_done else None

        # ---- schedule ----
        carry = phase_a()

        def wo_preload():
            load_wo(0)
            load_wo(1)
            yield

        # B0..B2: attn(sc) ⋈ [carry + ht(sc+1) (+ wo preload) + qkv(sc+1)]
        for sc in range(3):
            primary = attn_stream(sc)
            parts = [carry] if carry is not None else []
            parts += [load_ht(sc + 1)]
            if sc == 2:  # stage wo(0..1) early for phase C
                parts += [wo_preload()]
            parts += [qkv_stream(sc + 1)]
            n_carry = 4 if sc == 0 else 0
            filler = chain(*parts)
            n_primary = NQ * (4 * sc + 4)
            n_filler = n_carry + 6 + 14 * N_OB + (1 if sc == 2 else 0)
            carry = interleave(primary, filler, n_primary, n_filler,
                               reserve=14)
        if carry is not None:
            for _ in carry:
                pass

        # C: attn(3) ⋈ o_proj rows 0..11 (adaptive ratio: 78 filler units
        # must last all 96 primary slots or the attn tail runs PE-starved)
        primary = attn_stream(3)
        filler = oproj_c_stream()
        interleave(primary, filler, NQ * 16, N_MCP * 13)

        # D: o_proj rows 12..15
        oproj_d()

    nc.compile()
    return nc


def prep_inputs(positions, hidden_states, w_qkv, w_o):
    """Host-side shard + relayout. Returns per-core input maps."""
    bf = ml_dtypes.bfloat16
    pos = np.asarray(positions).astype(np.float32)
    hidden = np.ascontiguousarray(np.asarray(hidden_states, dtype=np.float32))
    w_qkv = np.asarray(w_qkv, dtype=np.float32)
    w_o = np.asarray(w_o, dtype=np.float32)

    # rope tables (neox): freqs [S, 64]
    inv_freq = 1.0 / (ROPE_THETA ** (np.arange(0, D, 2, dtype=np.float32) / D))
    freqs = pos[:, None] * inv_freq[None, :]
    cos = np.cos(freqs).T.astype(np.float32)   # [64, S]
    sin = np.sin(freqs).T.astype(np.float32)
    cosf = np.concatenate([cos, cos], axis=0).astype(bf)    # [128, S]
    sinf = np.concatenate([-sin, sin], axis=0).astype(bf)

    triu = np.triu(np.ones((128, 128), np.float32)).astype(bf)  # [k, q]: q >= k
    ones_col = np.ones((128, 1), np.float32).astype(bf)
    ident = np.eye(128, dtype=np.float32).astype(bf)

    # ht[sc, hbp, p, j*512+c] = hidden[sc*512+c, (2*hbp+j)*128+p]
    ht = np.ascontiguousarray(
        hidden.reshape(N_SC, SCW, N_HBP, 2, 128).transpose(0, 2, 4, 3, 1)
    ).reshape(N_SC, N_HBP, 128, 1024).astype(bf)

    in_maps = []
    for c in range(N_CORES):
        q_rows = w_qkv[c * NQ * D:(c + 1) * NQ * D]          # [768, 6144]
        k_rows = w_qkv[HID + c * D:HID + (c + 1) * D]        # [128, 6144]
        v_rows = w_qkv[HID + 8 * D + c * D:HID + 8 * D + (c + 1) * D]
        wq_c = np.concatenate([q_rows, k_rows, v_rows], axis=0)  # [1024, 6144]
        # wq[ob, p, hb, o] = wq_c[ob*128+o, hb*128+p]
        wq_arr = np.ascontiguousarray(
            wq_c.reshape(N_OB, 128, N_HB, 128).transpose(0, 3, 2, 1)).astype(bf)
        # wqa[g, pr, p, (i*4+m)*128+o] = wq_c[pair[pr][i]*128+o, (4g+m)*128+p]
        wq_blk = wq_c.reshape(N_OB, 128, 12, 4, 128)  # [ob, o, g, m, p]
        wqa_arr = np.empty((12, 4, 128, 1024), np.float32)
        for pr, obs in enumerate(((6, 7), (0, 1), (2, 3), (4, 5))):
            blk = wq_blk[list(obs)]                   # [i, o, g, m, p]
            wqa_arr[:, pr] = blk.transpose(2, 4, 0, 3, 1).reshape(12, 128, 1024)
        wqa_arr = np.ascontiguousarray(wqa_arr).astype(bf)
        wo_c = (w_o[:, c * NQ * D:(c + 1) * NQ * D] * ATTN_MULT).T  # [768, 6144]
        # wo[mc, p, fb, m] = wo_c[fb*128+p, mc*512+m]
        wo_arr = np.ascontiguousarray(
            wo_c.reshape(N_FB, 128, N_MC, SCW).transpose(2, 1, 0, 3)).astype(bf)
        in_maps.append({
            "ht": ht, "wq": wq_arr, "wqa": wqa_arr, "wo": wo_arr,
            "cosf": cosf, "sinf": sinf, "triu": triu,
            "ones_col": ones_col, "ident": ident,
            "negcap": np.full((128, 1), -SOFTCAP, np.float32),
        })
    return in_maps


_NC_CACHE = None


def _get_nc():
    global _NC_CACHE
    if _NC_CACHE is None:
        _NC_CACHE = build_nc()
    return _NC_CACHE


def kernel(positions, hidden_states, w_qkv, w_o, _trace=False, _trace_kwargs=None):
    nc = _get_nc()
    in_maps = prep_inputs(positions, hidden_states, w_qkv, w_o)
    res = run_bass_kernel_spmd(nc, in_maps, list(range(N_CORES)),
                               trace=_trace, **(_trace_kwargs or {}))
    out = np.zeros((S, HID), np.float32)
    for c in range(N_CORES):
        out += np.asarray(res.results[c]["out"]).astype(np.float32)
    out = out.astype(np.asarray(hidden_states).dtype)
    kernel.last_results = res
    return out


# revision 92
# speedup vs baseline: 1.0216x; 1.0049x over previous
"""Grok1-style GQA attention (S=2048, H=6144, 48 Q heads / 8 KV heads, rope,
softcap-30, causal) as a Bass/Tile kernel sharded over 8 NeuronCores.

Sharding: tensor-parallel across heads. Core c owns Q heads 6c..6c+5 and KV
head c. Each core computes its qkv projection slice, rope, causal softcap
attention for its 6 Q heads against its single KV head, and a partial
o_proj (its 768 columns of w_o). The host sums the 8 partial outputs.

Key numerics trick: softcap bounds scores to [-30, 30], so softmax is
computed as exp(30*tanh(s/30) - 30) with a *constant* bias — no running max.

Schedule (v2): the PE is ~91% busy at the bf16 roofline, so the focus is
removing the startup/tail stalls and improving DMA line efficiency.
  A : qkv(0) as ONE hb-major pass over all 8 output blocks (8 psum banks).
      The ht chunk is amortized over 4x more PE work than pair-wise passes,
      so DMA supply (~240 GB/s) stays ahead of PE consumption. wq arrives
      as (ob, qd) chunk tiles in consumption order on 2 queues.
  B0..B2: attn(sc) interleaved with [carry epilogues + ht(sc+1) + qkv(sc+1)]
  C : attn(3) interleaved with o_proj mc-pairs x s-rows 0..11
  D : o_proj s-rows 12..15, resident mc-pairs first
DMA line sizes: ht tiles pack hb-pairs ([128,1024] = 2KB lines), wo loads
one [128,3072] tile per mc (6KB lines), out writes [128,1024] mc-pair
tiles (2KB lines). wq chunks are [128,1536] (3KB lines).

Layouts (host-prepped, all transposed so the contraction dim is on SBUF
partitions):
  ht   [4,24,128,1024] bf16 : ht[sc,hbp,p,j*512+c] = hidden[sc*512+c,(2hbp+j)*128+p]
  wq   [8,128,48,128] bf16  : wq[ob,p,hb,o] = w_qkv_core[ob*128+o, hb*128+p]
  wo   [12,128,6,512] bf16  : wo[mc,p,fb,m] = (w_o[:,core]*MULT).T[fb*128+p, mc*512+m]
  cosf/sinf [128,2048] bf16 : duplicated/sign-flipped rope tables (neox)
  triu [128,128] bf16       : triu[k,q] = 1 if q >= k else 0
"""

import sys, os
import numpy as np

sys.path.insert(0, "/opt/trn_rl_repo")

import ml_dtypes

import concourse.bass as bass
import concourse.mybir as mybir
import concourse.tile as tile
from concourse import bacc
from concourse.bass_utils import run_bass_kernel_spmd

F32 = mybir.dt.float32
BF16 = mybir.dt.bfloat16
AF = mybir.ActivationFunctionType

S = 2048
HID = 6144
D = 128
NQ = 6          # q heads per core
N_CORES = 8
SCALE = D ** -0.5
SOFTCAP = 30.0
ATTN_MULT = 0.08838834764831845
ROPE_THETA = 10000.0

N_SC = 4        # s-chunks of 512
SCW = 512
N_HB = 48       # hidden 128-blocks
N_HBP = 24      # hidden 256-blocks (pairs)
N_OB = 8        # output 128-blocks per core (6 Q | 1 K | 1 V)
N_MC = 12       # o_proj 512-col chunks
N_MCP = 6       # o_proj 1024-col chunk pairs
N_SB = 16       # s 128-blocks
N_FB = 6        # per-core o_proj feature 128-blocks (768/128)

OB_ORDER = [6, 7, 0, 1, 2, 3, 4, 5]   # K,V first so epilogues unblock attn


def build_nc():
    nc = bacc.Bacc("TRN2", target_bir_lowering=False, debug=False, num_devices=N_CORES)

    ht_d = nc.dram_tensor("ht", [N_SC, N_HBP, 128, 1024], BF16, kind="ExternalInput").ap()
    wq_d = nc.dram_tensor("wq", [N_OB, 128, N_HB, 128], BF16, kind="ExternalInput").ap()
    # phase-A layout: [hb-group of 4][ob-pair][128 p][2 ob x 4 hb x 128 o]
    wqa_d = nc.dram_tensor("wqa", [12, 4, 128, 1024], BF16, kind="ExternalInput").ap()
    wo_d = nc.dram_tensor("wo", [N_MC, 128, N_FB, SCW], BF16, kind="ExternalInput").ap()
    cosf_d = nc.dram_tensor("cosf", [128, S], BF16, kind="ExternalInput").ap()
    sinf_d = nc.dram_tensor("sinf", [128, S], BF16, kind="ExternalInput").ap()
    triu_d = nc.dram_tensor("triu", [128, 128], BF16, kind="ExternalInput").ap()
    ones_col_d = nc.dram_tensor("ones_col", [128, 1], BF16, kind="ExternalInput").ap()
    ident_d = nc.dram_tensor("ident", [128, 128], BF16, kind="ExternalInput").ap()
    negcap_d = nc.dram_tensor("negcap", [128, 1], F32, kind="ExternalInput").ap()
    out_d = nc.dram_tensor("out", [S, HID], BF16, kind="ExternalOutput").ap()

    from contextlib import ExitStack
    with tile.TileContext(nc) as tc, ExitStack() as ctx:
        const = ctx.enter_context(tc.tile_pool(name="const", bufs=1))
        ktp = ctx.enter_context(tc.tile_pool(name="ktp", bufs=4))
        vnp = ctx.enter_context(tc.tile_pool(name="vnp", bufs=4))
        aotp = ctx.enter_context(tc.tile_pool(name="aotp", bufs=24))
        qtp = ctx.enter_context(tc.tile_pool(name="qtp", bufs=11))
        vtp = ctx.enter_context(tc.tile_pool(name="vtp", bufs=1))
        htp = ctx.enter_context(tc.tile_pool(name="htp", bufs=24))
        wqp = ctx.enter_context(tc.tile_pool(name="wqp", bufs=8))
        wqa = ctx.enter_context(tc.tile_pool(name="wqa", bufs=14))
        wop = ctx.enter_context(tc.tile_pool(name="wop", bufs=5))
        ropep = ctx.enter_context(tc.tile_pool(name="ropep", bufs=1))
        tpool = ctx.enter_context(tc.tile_pool(name="tpool", bufs=2))
        ppool = ctx.enter_context(tc.tile_pool(name="ppool", bufs=3))
        rpool = ctx.enter_context(tc.tile_pool(name="rpool", bufs=1))
        bpool = ctx.enter_context(tc.tile_pool(name="bpool", bufs=1))
        outp = ctx.enter_context(tc.tile_pool(name="outp", bufs=3))
        ps_a = ctx.enter_context(tc.tile_pool(name="ps_a", bufs=2, space=bass.MemorySpace.PSUM))
        ps_s = ctx.enter_context(tc.tile_pool(name="ps_s", bufs=2, space=bass.MemorySpace.PSUM))
        ps_pv = ctx.enter_context(tc.tile_pool(name="ps_pv", bufs=2, space=bass.MemorySpace.PSUM))
        ps_o = ctx.enter_context(tc.tile_pool(name="ps_o", bufs=2, space=bass.MemorySpace.PSUM))

        cosf = const.tile([128, S], BF16, tag="cosf", name="cosf")
        sinf = const.tile([128, S], BF16, tag="sinf", name="sinf")
        triu = const.tile([128, 128], BF16, tag="triu", name="triu")
        ones_col = const.tile([128, 1], BF16, tag="ones_col", name="ones_col")
        ident = const.tile([128, 128], BF16, tag="ident", name="ident")
        negcap = const.tile([128, 1], F32, tag="negcap", name="negcap")

        # wq chunk tiles: key (sc, ob, qd) -> [128, 12*128]
        wq_pref = {}

        def load_wq_chunk(sc, ob, qd, eng):
            t = wqp.tile([128, 12 * 128], BF16, tag="wq", name="wq")
            eng.dma_start(t[:], wq_d[ob, :, qd * 12:(qd + 1) * 12])
            wq_pref[(sc, ob, qd)] = t

        def load_wq_ob(sc, ob):
            """Stage all 4 qd chunks of one ob (only used when the ring is
            empty, so the triggers never block)."""
            for qd in range(4):
                load_wq_chunk(sc, ob, qd, nc.scalar)

        # per-chunk persistent tiles, filled as the pipeline progresses
        KT = {}    # sc -> [128, 512] bf16   (k^T, d on partitions)
        VN = {}    # sc -> [128, 512] bf16   (v natural, k on partitions)
        QT = {}    # (sc, h) -> [128, 512] bf16
        AOT = {}   # (sc, h) -> [128, 512] bf16
        ht_tiles = {}   # (sc, hbp) -> [128, 1024]

        def load_ht(sc):
            """Generator: issue ht hb-pair DMAs, 4 tiles per unit, rotating
            over all three DMA queues."""
            for hbp0 in range(0, N_HBP, 4):
                for hbp in range(hbp0, hbp0 + 4):
                    t = htp.tile([128, 1024], BF16, tag="ht", name="ht")
                    nc.sync.dma_start(t[:], ht_d[sc, hbp])
                    ht_tiles[(sc, hbp)] = t
                yield

        def ht_rhs(sc, hb):
            j = hb % 2
            return ht_tiles[(sc, hb // 2)][:, j * 512:(j + 1) * 512]

        def rope_epilogue(sc, ob, ps):
            scs = slice(sc * SCW, (sc + 1) * SCW)
            rot = ropep.tile([128, SCW], F32, tag="rot", name="rot")
            nc.scalar.copy(rot[0:64, :], ps[64:128, :])
            nc.scalar.copy(rot[64:128, :], ps[0:64, :])
            t1 = ropep.tile([128, SCW], F32, tag="t1", name="t1")
            nc.vector.tensor_mul(t1[:], ps[:], cosf[:, scs])
            nc.vector.tensor_mul(rot[:], rot[:], sinf[:, scs])
            if ob < NQ:
                qt = qtp.tile([128, SCW], BF16, tag="qt", name="qt")
                QT[(sc, ob)] = qt
                nc.vector.tensor_add(qt[:], t1[:], rot[:])
            else:
                kt = ktp.tile([128, SCW], BF16, tag="kt", name="kt")
                KT[sc] = kt
                nc.vector.tensor_add(kt[:], t1[:], rot[:])

        def v_epilogue(sc, ps, tp_pool, tp_tag):
            vt = vtp.tile([128, SCW], BF16, tag="vt", name="vt")
            nc.vector.tensor_copy(vt[:], ps[:])
            vn = vnp.tile([128, SCW], BF16, tag="vn", name="vn")
            VN[sc] = vn
            for j in range(4):
                tps = tp_pool.tile([128, 128], BF16, tag=tp_tag, name="tps")
                nc.tensor.transpose(tps[:], vt[:, j * 128:(j + 1) * 128], ident[:])
                nc.vector.tensor_copy(vn[:, j * 128:(j + 1) * 128], tps[:])

        # ---- phase A: qkv(0), single hb-major pass over all 8 obs ----
        # wq arrives as [128,1024] tiles: one per (4-hb group, ob-pair), in
        # consumption order spread over three queues.  The 16-deep ring keeps
        # supply ~4 groups (~27us of PE work) ahead of consumption.
        wq_a = {}   # (g, pair) -> tile
        A_PAIR = {6: (0, 0), 7: (0, 1), 0: (1, 0), 1: (1, 1),
                  2: (2, 0), 3: (2, 1), 4: (3, 0), 5: (3, 1)}

        def wq_a_slice(ob, hb):
            pr, i = A_PAIR[ob]
            t = wq_a[(hb // 4, pr)]
            col = (i * 4 + hb % 4) * 128
            return t[:, col:col + 128]

        def load_wqa_group(g, prs=(0, 1, 2, 3)):
            engs = (nc.scalar, nc.gpsimd)
            for pr in prs:
                t = wqa.tile([128, 1024], BF16, tag="wqa", name="wqa")
                engs[(g * 4 + pr) % 2].dma_start(t[:], wqa_d[g, pr])
                wq_a[(g, pr)] = t

        def phase_a():
            """Returns the carry generator of leftover epilogues."""
            # Upfront: first 8 ht hb-pairs and 3 wq groups; the rest issue
            # group-by-group between matmuls so ring-slot waits always see
            # their slot's readers already emitted.
            ht_loader = load_ht(0)
            next(ht_loader)
            load_wqa_group(0)
            load_wqa_group(1)
            next(ht_loader)
            load_wqa_group(2)
            load_wqa_group(3)

            # 8 psum accumulators: K,V -> ps_pv; q0,q1 -> ps_s; q2,q3 -> ps_a;
            # q4,q5 -> ps_o.  Tags reuse each pool's standard tag (pools ring
            # per tag).  Freed in that tail order for B0's needs.
            pools = {6: (ps_pv, "pv"), 7: (ps_pv, "pv"), 0: (ps_s, "s"),
                     1: (ps_s, "s"), 2: (ps_a, "acc"), 3: (ps_a, "acc"),
                     4: (ps_o, "oa"), 5: (ps_o, "oa")}
            acc = {}
            for ob in OB_ORDER:
                pool, tg = pools[ob]
                acc[ob] = pool.tile([128, SCW], F32, tag=tg, name=tg)

            def mm(ob, hb):
                nc.tensor.matmul(
                    acc[ob][:], lhsT=wq_a_slice(ob, hb), rhs=ht_rhs(0, hb),
                    start=(hb == 0), stop=(hb == N_HB - 1),
                )

            ht_units_at = {8: 1, 16: 1, 24: 1, 32: 1}
            for hb in range(44):
                # group g+3 prefetch, split so each tile's ring slot already
                # has its readers emitted (14-deep ring)
                if hb >= 4 and hb % 4 == 0 and hb // 4 + 3 <= 11:
                    load_wqa_group(hb // 4 + 3, prs=(0, 1))
                if hb >= 4 and hb % 4 == 2 and (hb - 2) // 4 + 3 <= 11:
                    load_wqa_group((hb - 2) // 4 + 3, prs=(2, 3))
                for _ in range(ht_units_at.get(hb, 0)):
                    next(ht_loader)
                if hb == 20:
                    # rope tables needed at the epilogues (~80us in); sync's
                    # ht(0) traffic is nearly drained by then, while scalar/
                    # gpsimd still carry the late wqa groups
                    nc.sync.dma_start(cosf[:], cosf_d[:])
                    nc.sync.dma_start(sinf[:], sinf_d[:])
                    nc.sync.dma_start(ident[:], ident_d[:])
                    nc.sync.dma_start(triu[:], triu_d[:])
                    nc.sync.dma_start(ones_col[:], ones_col_d[:])
                    nc.sync.dma_start(negcap[:], negcap_d[:])

                for ob in OB_ORDER:
                    mm(ob, hb)
            # staggered tail: finish K,V first, then q0,q1, so their
            # epilogue chains (scalar/vector) overlap the remaining tails.
            for hb in range(44, 48):
                mm(6, hb)
                mm(7, hb)
            rope_epilogue(0, 6, acc[6])
            vt = vtp.tile([128, SCW], BF16, tag="vt", name="vt")
            nc.vector.tensor_copy(vt[:], acc[7][:])
            for hb in range(44, 48):
                mm(0, hb)
                mm(1, hb)
            rope_epilogue(0, 0, acc[0])
            rope_epilogue(0, 1, acc[1])
            for hb in range(44, 48):
                mm(2, hb)
                mm(3, hb)
            vn = vnp.tile([128, SCW], BF16, tag="vn", name="vn")
            VN[0] = vn
            for j in range(4):
                tps = ps_pv.tile([128, 128], BF16, tag="pv", name="tps")
                nc.tensor.transpose(tps[:], vt[:, j * 128:(j + 1) * 128], ident[:])
                nc.vector.tensor_copy(vn[:, j * 128:(j + 1) * 128], tps[:])
            for hb in range(44, 48):
                mm(4, hb)
                mm(5, hb)
            wq_a.clear()
            load_wq_ob(1, OB_ORDER[0])  # stage qkv(1)'s first ob for B0

            def carry_gen():
                for ob in (2, 3, 4, 5):
                    rope_epilogue(0, ob, acc[ob])
                    yield
            return carry_gen()

        def qkv_stream(sc):
            """Generator: qkv projection + rope for chunk sc (1..3). Yields at
            boundaries where attention work may be interleaved. The next ob's
            wq chunks issue one-by-one right after the emissions that free
            their ring slot, so the (engine-blocking) DMA triggers never sit
            on a long slot wait in front of latency-critical scalar work."""
            # chunk qd issues after hb-group g of the current ob; gpsimd is
            # kept free of wq triggers so partition_broadcast (on the softmax
            # denominator critical chain) is never queued behind them.
            PREF = {2: (0, nc.scalar), 5: (1, nc.sync),
                    8: (2, nc.scalar), 11: (3, nc.scalar)}
            for idx, ob in enumerate(OB_ORDER):
                if idx + 1 < N_OB:
                    nxt = (sc, OB_ORDER[idx + 1])
                elif sc < 3:
                    nxt = (sc + 1, OB_ORDER[0])
                else:
                    nxt = None
                yield
                chunks = [wq_pref.pop((sc, ob, qd)) for qd in range(4)]
                ps = ps_a.tile([128, SCW], F32, tag="acc", name="acc")
                for hb0 in range(0, N_HB, 4):
                    for hb in range(hb0, hb0 + 4):
                        w = chunks[hb // 12]
                        nc.tensor.matmul(
                            ps[:],
                            lhsT=w[:, (hb % 12) * 128:(hb % 12 + 1) * 128],
                            rhs=ht_rhs(sc, hb),
                            start=(hb == 0),
                            stop=(hb == N_HB - 1),
                        )
                    if nxt is not None and hb0 // 4 in PREF:
                        qd, eng = PREF[hb0 // 4]
                        load_wq_chunk(nxt[0], nxt[1], qd, eng)
                    yield
                if ob <= NQ:
                    rope_epilogue(sc, ob, ps)
                else:
                    v_epilogue(sc, ps, ps_a, 'acc')
                yield

        def attn_stream(qc):
            """Generator: attention for q-chunk qc, all 6 heads. Score matmuls
            run LOOK iterations ahead of PV; normalization is deferred one
            head so recip/broadcast never block the vector engine's triu."""
            nkb = 4 * qc + 4
            iters = [(h, kb) for h in range(NQ) for kb in range(nkb)]
            n = len(iters)
            LOOK = 2
            state = {}
            pv_cur = {}
            oa_cur = {}
            pend = []   # deferred (pv, bc, h) normalizations

            def issue_score(idx):
                h, kb = iters[idx]
                qs = max(qc * SCW, kb * 128)
                off = qs - qc * SCW
                w = SCW - off
                sp = ps_s.tile([128, SCW], F32, tag="s", name="s")
                nc.tensor.matmul(
                    sp[:, :w],
                    lhsT=KT[kb // 4][:, (kb % 4) * 128:(kb % 4 + 1) * 128],
                    rhs=QT[(qc, h)][:, off:SCW],
                    start=True, stop=True,
                )
                tt = tpool.tile([128, SCW], F32, tag="t", name="t")
                nc.scalar.activation(tt[:, :w], sp[:, :w], AF.Tanh,
                                     scale=SCALE / SOFTCAP)
                pt = ppool.tile([128, SCW], BF16, tag="p", name="p")
                nc.scalar.activation(pt[:, :w], tt[:, :w], AF.Exp,
                                     scale=SOFTCAP, bias=negcap[:])
                if kb >= 4 * qc:
                    nc.vector.tensor_mul(pt[:, 0:128], pt[:, 0:128], triu[:])
                state[idx] = (pt, w, off)

            def flush_norm():
                pv, bc, h = pend.pop(0)
                at = aotp.tile([128, SCW], BF16, tag="aot", name="aot")
                AOT[(qc, h)] = at
                nc.vector.tensor_mul(at[:], pv[:], bc[:])

            def issue_pv(idx):
                h, kb = iters[idx]
                pt, w, off = state.pop(idx)
                if kb == 0:
                    pv_cur[h] = ps_pv.tile([128, SCW], F32, tag="pv", name="pv")
                    oa_cur[h] = ps_o.tile([1, SCW], F32, tag="oa", name="oa")
                if kb == 1 and pend:
                    flush_norm()
                pv, oa = pv_cur[h], oa_cur[h]
                nc.tensor.matmul(
                    pv[:, off:SCW],
                    lhsT=VN[kb // 4][:, (kb % 4) * 128:(kb % 4 + 1) * 128],
                    rhs=pt[:, :w],
                    start=(kb == 0), stop=(kb == nkb - 1),
                )
                nc.tensor.matmul(
                    oa[0:1, off:SCW],
                    lhsT=ones_col[:],
                    rhs=pt[:, :w],
                    start=(kb == 0), stop=(kb == nkb - 1),
                )
                if kb == nkb - 1:
                    rr = rpool.tile([1, SCW], F32, tag="r", name="r")
                    # [1,512] single-partition DVE op: plain reciprocal costs
                    # 3.3us on the head-boundary critical chain; approx_fast
                    # (~18 bits, den is a positive normal) is 5x faster.
                    nc.vector.reciprocal_approx_fast(rr[:], oa[0:1, :])
                    bc = bpool.tile([128, SCW], F32, tag="bc", name="bc")
                    nc.gpsimd.partition_broadcast(bc[:], rr[:])
                    pend.append((pv, bc, h))

            for j in range(min(LOOK, n)):
                issue_score(j)
            for i in range(n):
                if i + LOOK < n:
                    issue_score(i + LOOK)
                yield
                issue_pv(i)
            while pend:
                flush_norm()

        # ---- o_proj: mc-pair granularity, [128,1024] out tiles ----
        wo_tiles = {}

        def load_wo(mc, eng=None):
            wos = wop.tile([128, N_FB * SCW], BF16, tag="wo", name="wo")
            (eng or nc.sync).dma_start(wos[:], wo_d[mc])
            wo_tiles[mc] = wos

        def oproj_row(sb, mcp, wr_eng=None, split_wr=False):
            """One [128,1024] output tile: s-block sb x mc pair (2mcp, 2mcp+1)."""
            sc, j = sb // 4, sb % 4
            ot = outp.tile([128, 1024], BF16, tag="out", name="out")
            for half in range(2):
                mc = 2 * mcp + half
                wos = wo_tiles[mc]
                op = ps_a.tile([128, SCW], F32, tag="acc", name="acc")
                for fb in range(N_FB):
                    nc.tensor.matmul(
                        op[:],
                        lhsT=AOT[(sc, fb)][:, j * 128:(j + 1) * 128],
                        rhs=wos[:, fb * SCW:(fb + 1) * SCW],
                        start=(fb == 0), stop=(fb == N_FB - 1),
                    )
                if half == 0:
                    nc.vector.tensor_copy(ot[:, 0:512], op[:])
                    if split_wr:  # drain the tail write in parallel halves
                        nc.scalar.dma_start(
                            out_d[sb * 128:(sb + 1) * 128,
                                  mcp * 1024:mcp * 1024 + 512], ot[:, 0:512])
                else:
                    nc.scalar.copy(ot[:, 512:1024], op[:])
            if split_wr:
                nc.sync.dma_start(
                    out_d[sb * 128:(sb + 1) * 128,
                          mcp * 1024 + 512:(mcp + 1) * 1024], ot[:, 512:1024])
            else:
                (wr_eng or nc.sync).dma_start(
                    out_d[sb * 128:(sb + 1) * 128, mcp * 1024:(mcp + 1) * 1024], ot[:])

        def oproj_c_stream():
            """Phase C: mcp-outer over s-rows 0..11. wo(0,1) preloaded in B2;
            pairs load just in time on gpsimd (sync carries the out writes);
            ring keeps the last 6 mc for phase D."""
            for mcp in range(N_MCP):
                if mcp + 1 < N_MCP:
                    load_wo(2 * mcp + 2, nc.gpsimd)
                    load_wo(2 * mcp + 3, nc.gpsimd)
                yield
                for sb in range(12):
                    oproj_row(sb, mcp)
                    yield

        def oproj_d():
            """Phase D: s-rows 12..15. mc 7..11 still in the wo ring (bufs=5);
            mc 0..5 reload staggered behind the resident mcps' rows. Writes
            alternate scalar/sync; wo pairs split gpsimd/sync."""
            def rows(mcp, last=False):
                for sb in range(12, 16):
                    oproj_row(sb, mcp, wr_eng=nc.scalar if sb % 2 else nc.sync,
                              split_wr=(last and sb == 15))
            rows(4)
            load_wo(6, nc.gpsimd)
            load_wo(7, nc.sync)
            rows(5)
            load_wo(0, nc.gpsimd)
            load_wo(1, nc.sync)
            rows(3)
            load_wo(2, nc.gpsimd)
            load_wo(3, nc.sync)
            rows(0)
            load_wo(4, nc.gpsimd)
            load_wo(5, nc.sync)
            rows(1)
            rows(2, last=True)

        def chain(*gens):
            for g in gens:
                yield from g

        def interleave(primary, filler, n_primary, n_filler, reserve=0,
                       ratio=None, drain=True):
            """Advance primary; between slots advance filler so both streams
            finish together (adaptive, or fixed `ratio`). Keep `reserve`
            filler units unexecuted; drain (or hand back) the remainder."""
            rem_p, rem_f = n_primary, n_filler
            acc = 0.0
            f_done = False
            for _ in primary:
                rem_p -= 1
                if not f_done:
                    acc += ratio if ratio is not None else rem_f / max(rem_p, 1)
                    while acc >= 1.0 and not f_done and rem_f > reserve:
                        try:
                            next(filler)
                            rem_f -= 1
                        except StopIteration:
                            f_done = True
                        acc -= 1.0
            if drain and not f_done:
                for _ in filler:
                    pass
            return filler if not f_done else None

        # ---- schedule ----
        carry = phase_a()

        def wo_preload():
            load_wo(0)
            load_wo(1)
            yield

        # B0..B2: attn(sc) ⋈ [carry + ht(sc+1) (+ wo preload) + qkv(sc+1)]
        for sc in range(3):
            primary = attn_stream(sc)
            parts = [carry] if carry is not None else []
            parts += [load_ht(sc + 1)]
            if sc == 2:  # stage wo(0..1) early for phase C
                parts += [wo_preload()]
            parts += [qkv_stream(sc + 1)]
            n_carry = 4 if sc == 0 else 0
            filler = chain(*parts)
            n_primary = NQ * (4 * sc + 4)
            n_filler = n_carry + 6 + 14 * N_OB + (1 if sc == 2 else 0)
            carry = interleave(primary, filler, n_primary, n_filler,
                               reserve=14)
        if carry is not None:
            for _ in carry:
                pass

        # C: attn(3) ⋈ o_proj rows 0..11 (adaptive ratio: 78 filler units
        # must last all 96 primary slots or the attn tail runs PE-starved)
        primary = attn_stream(3)
        filler = oproj_c_stream()
        interleave(primary, filler, NQ * 16, N_MCP * 13)

        # D: o_proj rows 12..15
        oproj_d()

    nc.compile()
    return nc


def prep_inputs(positions, hidden_states, w_qkv, w_o):
    """Host-side shard + relayout. Returns per-core input maps."""
    bf = ml_dtypes.bfloat16
    pos = np.asarray(positions).astype(np.float32)
    hidden = np.ascontiguousarray(np.asarray(hidden_states, dtype=np.float32))
    w_qkv = np.asarray(w_qkv, dtype=np.float32)
    w_o = np.asarray(w_o, dtype=np.float32)

    # rope tables (neox): freqs [S, 64]
    inv_freq = 1.0 / (ROPE_THETA ** (np.arange(0, D, 2, dtype=np.float32) / D))
    freqs = pos[:, None] * inv_freq[None, :]
    cos = np.cos(freqs).T.astype(np.float32)   # [64, S]
    sin = np.sin(freqs).T.astype(np.float32)
    cosf = np.concatenate([cos, cos], axis=0).astype(bf)    # [128, S]
    sinf = np.concatenate([-sin, sin], axis=0).astype(bf)

    triu = np.triu(np.ones((128, 128), np.float32)).astype(bf)  # [k, q]: q >= k
    ones_col = np.ones((128, 1), np.float32).astype(bf)
    ident = np.eye(128, dtype=np.float32).astype(bf)

    # ht[sc, hbp, p, j*512+c] = hidden[sc*512+c, (2*hbp+j)*128+p]
    ht = np.ascontiguousarray(
        hidden.reshape(N_SC, SCW, N_HBP, 2, 128).transpose(0, 2, 4, 3, 1)
    ).reshape(N_SC, N_HBP, 128, 1024).astype(bf)

    in_maps = []
    for c in range(N_CORES):
        q_rows = w_qkv[c * NQ * D:(c + 1) * NQ * D]          # [768, 6144]
        k_rows = w_qkv[HID + c * D:HID + (c + 1) * D]        # [128, 6144]
        v_rows = w_qkv[HID + 8 * D + c * D:HID + 8 * D + (c + 1) * D]
        wq_c = np.concatenate([q_rows, k_rows, v_rows], axis=0)  # [1024, 6144]
        # wq[ob, p, hb, o] = wq_c[ob*128+o, hb*128+p]
        wq_arr = np.ascontiguousarray(
            wq_c.reshape(N_OB, 128, N_HB, 128).transpose(0, 3, 2, 1)).astype(bf)
        # wqa[g, pr, p, (i*4+m)*128+o] = wq_c[pair[pr][i]*128+o, (4g+m)*128+p]
        wq_blk = wq_c.reshape(N_OB, 128, 12, 4, 128)  # [ob, o, g, m, p]
        wqa_arr = np.empty((12, 4, 128, 1024), np.float32)
        for pr, obs in enumerate(((6, 7), (0, 1), (2, 3), (4, 5))):
            blk = wq_blk[list(obs)]                   # [i, o, g, m, p]
            wqa_arr[:, pr] = blk.transpose(2, 4, 0, 3, 1).reshape(12, 128, 1024)
        wqa_arr = np.ascontiguousarray(wqa_arr).astype(bf)
        wo_c = (w_o[:, c * NQ * D:(c + 1) * NQ * D] * ATTN_MULT).T  # [768, 6144]
        # wo[mc, p, fb, m] = wo_c[fb*128+p, mc*512+m]
        wo_arr = np.ascontiguousarray(
            wo_c.reshape(N_FB, 128, N_MC, SCW).transpose(2, 1, 0, 3)).astype(bf)
        in_maps.append({
            "ht": ht, "wq": wq_arr, "wqa": wqa_arr, "wo": wo_arr,
            "cosf": cosf, "sinf": sinf, "triu": triu,
            "ones_col": ones_col, "ident": ident,
            "negcap": np.full((128, 1), -SOFTCAP, np.float32),
        })
    return in_maps


_NC_CACHE = None


def _get_nc():
    global _NC_CACHE
    if _NC_CACHE is None:
        _NC_CACHE = build_nc()
    return _NC_CACHE


def kernel(positions, hidden_states, w_qkv, w_o, _trace=False, _trace_kwargs=None):
    nc = _get_nc()
    in_maps = prep_inputs(positions, hidden_states, w_qkv, w_o)
    res = run_bass_kernel_spmd(nc, in_maps, list(range(N_CORES)),
                               trace=_trace, **(_trace_kwargs or {}))
    out = np.zeros((S, HID), np.float32)
    for c in range(N_CORES):
        out += np.asarray(res.results[c]["out"]).astype(np.float32)
    out = out.astype(np.asarray(hidden_states).dtype)
    kernel.last_results = res
    return out


# revision 94
# speedup vs baseline: 1.0265x; 1.0048x over previous
"""Grok1-style GQA attention (S=2048, H=6144, 48 Q heads / 8 KV heads, rope,
softcap-30, causal) as a Bass/Tile kernel sharded over 8 NeuronCores.

Sharding: tensor-parallel across heads. Core c owns Q heads 6c..6c+5 and KV
head c. Each core computes its qkv projection slice, rope, causal softcap
attention for its 6 Q heads against its single KV head, and a partial
o_proj (its 768 columns of w_o). The host sums the 8 partial outputs.

Key numerics trick: softcap bounds scores to [-30, 30], so softmax is
computed as exp(30*tanh(s/30) - 30) with a *constant* bias — no running max.

Schedule (v2): the PE is ~91% busy at the bf16 roofline, so the focus is
removing the startup/tail stalls and improving DMA line efficiency.
  A : qkv(0) as ONE hb-major pass over all 8 output blocks (8 psum banks).
      The ht chunk is amortized over 4x more PE work than pair-wise passes,
      so DMA supply (~240 GB/s) stays ahead of PE consumption. wq arrives
      as (ob, qd) chunk tiles in consumption order on 2 queues.
  B0..B2: attn(sc) interleaved with [carry epilogues + ht(sc+1) + qkv(sc+1)]
  C : attn(3) interleaved with o_proj mc-pairs x s-rows 0..11
  D : o_proj s-rows 12..15, resident mc-pairs first
DMA line sizes: ht tiles pack hb-pairs ([128,1024] = 2KB lines), wo loads
one [128,3072] tile per mc (6KB lines), out writes [128,1024] mc-pair
tiles (2KB lines). wq chunks are [128,1536] (3KB lines).

Layouts (host-prepped, all transposed so the contraction dim is on SBUF
partitions):
  ht   [4,24,128,1024] bf16 : ht[sc,hbp,p,j*512+c] = hidden[sc*512+c,(2hbp+j)*128+p]
  wq   [8,128,48,128] bf16  : wq[ob,p,hb,o] = w_qkv_core[ob*128+o, hb*128+p]
  wo   [12,128,6,512] bf16  : wo[mc,p,fb,m] = (w_o[:,core]*MULT).T[fb*128+p, mc*512+m]
  cosf/sinf [128,2048] bf16 : duplicated/sign-flipped rope tables (neox)
  triu [128,128] bf16       : triu[k,q] = 1 if q >= k else 0
"""

import sys, os
import numpy as np

sys.path.insert(0, "/opt/trn_rl_repo")

import ml_dtypes

import concourse.bass as bass
import concourse.mybir as mybir
import concourse.tile as tile
from concourse import bacc
from concourse.bass_utils import run_bass_kernel_spmd

F32 = mybir.dt.float32
BF16 = mybir.dt.bfloat16
AF = mybir.ActivationFunctionType

S = 2048
HID = 6144
D = 128
NQ = 6          # q heads per core
N_CORES = 8
SCALE = D ** -0.5
SOFTCAP = 30.0
ATTN_MULT = 0.08838834764831845
ROPE_THETA = 10000.0

N_SC = 4        # s-chunks of 512
SCW = 512
N_HB = 48       # hidden 128-blocks
N_HBP = 24      # hidden 256-blocks (pairs)
N_OB = 8        # output 128-blocks per core (6 Q | 1 K | 1 V)
N_MC = 12       # o_proj 512-col chunks
N_MCP = 6       # o_proj 1024-col chunk pairs
N_SB = 16       # s 128-blocks
N_FB = 6        # per-core o_proj feature 128-blocks (768/128)

OB_ORDER = [6, 7, 0, 1, 2, 3, 4, 5]   # K,V first so epilogues unblock attn


def build_nc():
    nc = bacc.Bacc("TRN2", target_bir_lowering=False, debug=False, num_devices=N_CORES)

    ht_d = nc.dram_tensor("ht", [N_SC, N_HBP, 128, 1024], BF16, kind="ExternalInput").ap()
    wq_d = nc.dram_tensor("wq", [N_OB, 128, N_HB, 128], BF16, kind="ExternalInput").ap()
    # phase-A layout: [hb-group of 4][ob-pair][128 p][2 ob x 4 hb x 128 o]
    wqa_d = nc.dram_tensor("wqa", [12, 4, 128, 1024], BF16, kind="ExternalInput").ap()
    wo_d = nc.dram_tensor("wo", [N_MC, 128, N_FB, SCW], BF16, kind="ExternalInput").ap()
    cosf_d = nc.dram_tensor("cosf", [128, S], BF16, kind="ExternalInput").ap()
    sinf_d = nc.dram_tensor("sinf", [128, S], BF16, kind="ExternalInput").ap()
    triu_d = nc.dram_tensor("triu", [128, 128], BF16, kind="ExternalInput").ap()
    ones_col_d = nc.dram_tensor("ones_col", [128, 1], BF16, kind="ExternalInput").ap()
    ident_d = nc.dram_tensor("ident", [128, 128], BF16, kind="ExternalInput").ap()
    negcap_d = nc.dram_tensor("negcap", [128, 1], F32, kind="ExternalInput").ap()
    out_d = nc.dram_tensor("out", [S, HID], BF16, kind="ExternalOutput").ap()

    from contextlib import ExitStack
    with tile.TileContext(nc) as tc, ExitStack() as ctx:
        const = ctx.enter_context(tc.tile_pool(name="const", bufs=1))
        ktp = ctx.enter_context(tc.tile_pool(name="ktp", bufs=4))
        vnp = ctx.enter_context(tc.tile_pool(name="vnp", bufs=4))
        aotp = ctx.enter_context(tc.tile_pool(name="aotp", bufs=24))
        qtp = ctx.enter_context(tc.tile_pool(name="qtp", bufs=11))
        vtp = ctx.enter_context(tc.tile_pool(name="vtp", bufs=1))
        htp = ctx.enter_context(tc.tile_pool(name="htp", bufs=24))
        wqp = ctx.enter_context(tc.tile_pool(name="wqp", bufs=8))
        wqa = ctx.enter_context(tc.tile_pool(name="wqa", bufs=14))
        wop = ctx.enter_context(tc.tile_pool(name="wop", bufs=5))
        ropep = ctx.enter_context(tc.tile_pool(name="ropep", bufs=1))
        tpool = ctx.enter_context(tc.tile_pool(name="tpool", bufs=2))
        ppool = ctx.enter_context(tc.tile_pool(name="ppool", bufs=3))
        rpool = ctx.enter_context(tc.tile_pool(name="rpool", bufs=1))
        bpool = ctx.enter_context(tc.tile_pool(name="bpool", bufs=1))
        outp = ctx.enter_context(tc.tile_pool(name="outp", bufs=3))
        ps_a = ctx.enter_context(tc.tile_pool(name="ps_a", bufs=2, space=bass.MemorySpace.PSUM))
        ps_s = ctx.enter_context(tc.tile_pool(name="ps_s", bufs=2, space=bass.MemorySpace.PSUM))
        ps_pv = ctx.enter_context(tc.tile_pool(name="ps_pv", bufs=2, space=bass.MemorySpace.PSUM))
        ps_o = ctx.enter_context(tc.tile_pool(name="ps_o", bufs=2, space=bass.MemorySpace.PSUM))

        cosf = const.tile([128, S], BF16, tag="cosf", name="cosf")
        sinf = const.tile([128, S], BF16, tag="sinf", name="sinf")
        triu = const.tile([128, 128], BF16, tag="triu", name="triu")
        ones_col = const.tile([128, 1], BF16, tag="ones_col", name="ones_col")
        ident = const.tile([128, 128], BF16, tag="ident", name="ident")
        negcap = const.tile([128, 1], F32, tag="negcap", name="negcap")

        # wq chunk tiles: key (sc, ob, qd) -> [128, 12*128]
        wq_pref = {}

        def load_wq_chunk(sc, ob, qd, eng):
            t = wqp.tile([128, 12 * 128], BF16, tag="wq", name="wq")
            eng.dma_start(t[:], wq_d[ob, :, qd * 12:(qd + 1) * 12])
            wq_pref[(sc, ob, qd)] = t

        def load_wq_ob(sc, ob):
            """Stage all 4 qd chunks of one ob (only used when the ring is
            empty, so the triggers never block)."""
            for qd in range(4):
                load_wq_chunk(sc, ob, qd, nc.scalar)

        # per-chunk persistent tiles, filled as the pipeline progresses
        KT = {}    # sc -> [128, 512] bf16   (k^T, d on partitions)
        VN = {}    # sc -> [128, 512] bf16   (v natural, k on partitions)
        QT = {}    # (sc, h) -> [128, 512] bf16
        AOT = {}   # (sc, h) -> [128, 512] bf16
        ht_tiles = {}   # (sc, hbp) -> [128, 1024]

        def load_ht(sc):
            """Generator: issue ht hb-pair DMAs, 4 tiles per unit, rotating
            over all three DMA queues."""
            for hbp0 in range(0, N_HBP, 4):
                for hbp in range(hbp0, hbp0 + 4):
                    t = htp.tile([128, 1024], BF16, tag="ht", name="ht")
                    nc.sync.dma_start(t[:], ht_d[sc, hbp])
                    ht_tiles[(sc, hbp)] = t
                yield

        def ht_rhs(sc, hb):
            j = hb % 2
            return ht_tiles[(sc, hb // 2)][:, j * 512:(j + 1) * 512]

        def rope_epilogue(sc, ob, ps):
            scs = slice(sc * SCW, (sc + 1) * SCW)
            rot = ropep.tile([128, SCW], F32, tag="rot", name="rot")
            nc.scalar.copy(rot[0:64, :], ps[64:128, :])
            nc.scalar.copy(rot[64:128, :], ps[0:64, :])
            t1 = ropep.tile([128, SCW], F32, tag="t1", name="t1")
            nc.vector.tensor_mul(t1[:], ps[:], cosf[:, scs])
            nc.vector.tensor_mul(rot[:], rot[:], sinf[:, scs])
            if ob < NQ:
                qt = qtp.tile([128, SCW], BF16, tag="qt", name="qt")
                QT[(sc, ob)] = qt
                nc.vector.tensor_add(qt[:], t1[:], rot[:])
            else:
                kt = ktp.tile([128, SCW], BF16, tag="kt", name="kt")
                KT[sc] = kt
                nc.vector.tensor_add(kt[:], t1[:], rot[:])

        def v_epilogue(sc, ps, tp_pool, tp_tag):
            vt = vtp.tile([128, SCW], BF16, tag="vt", name="vt")
            nc.vector.tensor_copy(vt[:], ps[:])
            vn = vnp.tile([128, SCW], BF16, tag="vn", name="vn")
            VN[sc] = vn
            for j in range(4):
                tps = tp_pool.tile([128, 128], BF16, tag=tp_tag, name="tps")
                nc.tensor.transpose(tps[:], vt[:, j * 128:(j + 1) * 128], ident[:])
                nc.vector.tensor_copy(vn[:, j * 128:(j + 1) * 128], tps[:])

        # ---- phase A: qkv(0), single hb-major pass over all 8 obs ----
        # wq arrives as [128,1024] tiles: one per (4-hb group, ob-pair), in
        # consumption order spread over three queues.  The 16-deep ring keeps
        # supply ~4 groups (~27us of PE work) ahead of consumption.
        wq_a = {}   # (g, pair) -> tile
        A_PAIR = {6: (0, 0), 7: (0, 1), 0: (1, 0), 1: (1, 1),
                  2: (2, 0), 3: (2, 1), 4: (3, 0), 5: (3, 1)}

        def wq_a_slice(ob, hb):
            pr, i = A_PAIR[ob]
            t = wq_a[(hb // 4, pr)]
            col = (i * 4 + hb % 4) * 128
            return t[:, col:col + 128]

        def load_wqa_group(g, prs=(0, 1, 2, 3)):
            engs = (nc.scalar, nc.gpsimd)
            for pr in prs:
                t = wqa.tile([128, 1024], BF16, tag="wqa", name="wqa")
                engs[(g * 4 + pr) % 2].dma_start(t[:], wqa_d[g, pr])
                wq_a[(g, pr)] = t

        def phase_a():
            """Returns the carry generator of leftover epilogues."""
            # Upfront: first 8 ht hb-pairs and 3 wq groups; the rest issue
            # group-by-group between matmuls so ring-slot waits always see
            # their slot's readers already emitted.
            ht_loader = load_ht(0)
            next(ht_loader)
            load_wqa_group(0)
            load_wqa_group(1)
            next(ht_loader)
            load_wqa_group(2)
            load_wqa_group(3)

            # 8 psum accumulators: K,V -> ps_pv; q0,q1 -> ps_s; q2,q3 -> ps_a;
            # q4,q5 -> ps_o.  Tags reuse each pool's standard tag (pools ring
            # per tag).  Freed in that tail order for B0's needs.
            pools = {6: (ps_pv, "pv"), 7: (ps_pv, "pv"), 0: (ps_s, "s"),
                     1: (ps_s, "s"), 2: (ps_a, "acc"), 3: (ps_a, "acc"),
                     4: (ps_o, "oa"), 5: (ps_o, "oa")}
            acc = {}
            for ob in OB_ORDER:
                pool, tg = pools[ob]
                acc[ob] = pool.tile([128, SCW], F32, tag=tg, name=tg)

            def mm(ob, hb):
                nc.tensor.matmul(
                    acc[ob][:], lhsT=wq_a_slice(ob, hb), rhs=ht_rhs(0, hb),
                    start=(hb == 0), stop=(hb == N_HB - 1),
                )

            ht_units_at = {8: 1, 16: 1, 24: 1, 32: 1}
            for hb in range(44):
                # group g+3 prefetch, split so each tile's ring slot already
                # has its readers emitted (14-deep ring)
                if hb >= 4 and hb % 4 == 0 and hb // 4 + 3 <= 11:
                    load_wqa_group(hb // 4 + 3, prs=(0, 1))
                if hb >= 4 and hb % 4 == 2 and (hb - 2) // 4 + 3 <= 11:
                    load_wqa_group((hb - 2) // 4 + 3, prs=(2, 3))
                for _ in range(ht_units_at.get(hb, 0)):
                    next(ht_loader)
                if hb == 20:
                    # rope tables needed at the epilogues (~80us in); sync's
                    # ht(0) traffic is nearly drained by then, while scalar/
                    # gpsimd still carry the late wqa groups
                    nc.sync.dma_start(cosf[:], cosf_d[:])
                    nc.sync.dma_start(sinf[:], sinf_d[:])
                    nc.sync.dma_start(ident[:], ident_d[:])
                    nc.sync.dma_start(triu[:], triu_d[:])
                    nc.sync.dma_start(ones_col[:], ones_col_d[:])
                    nc.sync.dma_start(negcap[:], negcap_d[:])

                for ob in OB_ORDER:
                    mm(ob, hb)
            # staggered tail: finish K,V first, then q0,q1, so their
            # epilogue chains (scalar/vector) overlap the remaining tails.
            for hb in range(44, 48):
                mm(6, hb)
                mm(7, hb)
            rope_epilogue(0, 6, acc[6])
            vt = vtp.tile([128, SCW], BF16, tag="vt", name="vt")
            nc.vector.tensor_copy(vt[:], acc[7][:])
            for hb in range(44, 48):
                mm(0, hb)
                mm(1, hb)
            rope_epilogue(0, 0, acc[0])
            rope_epilogue(0, 1, acc[1])
            for hb in range(44, 48):
                mm(2, hb)
                mm(3, hb)
            vn = vnp.tile([128, SCW], BF16, tag="vn", name="vn")
            VN[0] = vn
            for j in range(4):
                tps = ps_pv.tile([128, 128], BF16, tag="pv", name="tps")
                nc.tensor.transpose(tps[:], vt[:, j * 128:(j + 1) * 128], ident[:])
                nc.vector.tensor_copy(vn[:, j * 128:(j + 1) * 128], tps[:])
            for hb in range(44, 48):
                mm(4, hb)
                mm(5, hb)
            wq_a.clear()
            # stage qkv(1)'s first ob + half the second for B0 (scalar's wqa
            # traffic is done by the time these transfer)
            load_wq_ob(1, OB_ORDER[0])
            load_wq_chunk(1, OB_ORDER[1], 0, nc.scalar)
            load_wq_chunk(1, OB_ORDER[1], 1, nc.scalar)

            def carry_gen():
                for ob in (2, 3, 4, 5):
                    rope_epilogue(0, ob, acc[ob])
                    yield
            return carry_gen()

        def qkv_stream(sc):
            """Generator: qkv projection + rope for chunk sc (1..3). Yields at
            boundaries where attention work may be interleaved. The next ob's
            wq chunks issue one-by-one right after the emissions that free
            their ring slot, so the (engine-blocking) DMA triggers never sit
            on a long slot wait in front of latency-critical scalar work."""
            # chunk qd issues after hb-group g of the current ob; gpsimd is
            # kept free of wq triggers so partition_broadcast (on the softmax
            # denominator critical chain) is never queued behind them.
            PREF = {2: (0, nc.scalar), 5: (1, nc.sync),
                    8: (2, nc.scalar), 11: (3, nc.scalar)}
            for idx, ob in enumerate(OB_ORDER):
                if idx + 1 < N_OB:
                    nxt = (sc, OB_ORDER[idx + 1])
                elif sc < 3:
                    nxt = (sc + 1, OB_ORDER[0])
                else:
                    nxt = None
                yield
                chunks = [wq_pref.pop((sc, ob, qd)) for qd in range(4)]
                ps = ps_a.tile([128, SCW], F32, tag="acc", name="acc")
                for hb0 in range(0, N_HB, 4):
                    for hb in range(hb0, hb0 + 4):
                        w = chunks[hb // 12]
                        nc.tensor.matmul(
                            ps[:],
                            lhsT=w[:, (hb % 12) * 128:(hb % 12 + 1) * 128],
                            rhs=ht_rhs(sc, hb),
                            start=(hb == 0),
                            stop=(hb == N_HB - 1),
                        )
                    if nxt is not None and hb0 // 4 in PREF:
                        qd, eng = PREF[hb0 // 4]
                        if (nxt[0], nxt[1], qd) not in wq_pref:
                            load_wq_chunk(nxt[0], nxt[1], qd, eng)
                    yield
                if ob <= NQ:
                    rope_epilogue(sc, ob, ps)
                else:
                    v_epilogue(sc, ps, ps_a, 'acc')
                yield

        def attn_stream(qc):
            """Generator: attention for q-chunk qc, all 6 heads. Score matmuls
            run LOOK iterations ahead of PV; normalization is deferred one
            head so recip/broadcast never block the vector engine's triu."""
            nkb = 4 * qc + 4
            iters = [(h, kb) for h in range(NQ) for kb in range(nkb)]
            n = len(iters)
            LOOK = 2
            state = {}
            pv_cur = {}
            oa_cur = {}
            pend = []   # deferred (pv, bc, h) normalizations

            def issue_score(idx):
                h, kb = iters[idx]
                qs = max(qc * SCW, kb * 128)
                off = qs - qc * SCW
                w = SCW - off
                sp = ps_s.tile([128, SCW], F32, tag="s", name="s")
                nc.tensor.matmul(
                    sp[:, :w],
                    lhsT=KT[kb // 4][:, (kb % 4) * 128:(kb % 4 + 1) * 128],
                    rhs=QT[(qc, h)][:, off:SCW],
                    start=True, stop=True,
                )
                tt = tpool.tile([128, SCW], F32, tag="t", name="t")
                nc.scalar.activation(tt[:, :w], sp[:, :w], AF.Tanh,
                                     scale=SCALE / SOFTCAP)
                pt = ppool.tile([128, SCW], BF16, tag="p", name="p")
                nc.scalar.activation(pt[:, :w], tt[:, :w], AF.Exp,
                                     scale=SOFTCAP, bias=negcap[:])
                if kb >= 4 * qc:
                    nc.vector.tensor_mul(pt[:, 0:128], pt[:, 0:128], triu[:])
                state[idx] = (pt, w, off)

            def flush_norm():
                pv, bc, h = pend.pop(0)
                at = aotp.tile([128, SCW], BF16, tag="aot", name="aot")
                AOT[(qc, h)] = at
                nc.vector.tensor_mul(at[:], pv[:], bc[:])

            def issue_pv(idx):
                h, kb = iters[idx]
                pt, w, off = state.pop(idx)
                if kb == 0:
                    pv_cur[h] = ps_pv.tile([128, SCW], F32, tag="pv", name="pv")
                    oa_cur[h] = ps_o.tile([1, SCW], F32, tag="oa", name="oa")
                if kb == 1 and pend:
                    flush_norm()
                pv, oa = pv_cur[h], oa_cur[h]
                nc.tensor.matmul(
                    pv[:, off:SCW],
                    lhsT=VN[kb // 4][:, (kb % 4) * 128:(kb % 4 + 1) * 128],
                    rhs=pt[:, :w],
                    start=(kb == 0), stop=(kb == nkb - 1),
                )
                nc.tensor.matmul(
                    oa[0:1, off:SCW],
                    lhsT=ones_col[:],
                    rhs=pt[:, :w],
                    start=(kb == 0), stop=(kb == nkb - 1),
                )
                if kb == nkb - 1:
                    rr = rpool.tile([1, SCW], F32, tag="r", name="r")
                    # [1,512] single-partition DVE op: plain reciprocal costs
                    # 3.3us on the head-boundary critical chain; approx_fast
                    # (~18 bits, den is a positive normal) is 5x faster.
                    nc.vector.reciprocal_approx_fast(rr[:], oa[0:1, :])
                    bc = bpool.tile([128, SCW], F32, tag="bc", name="bc")
                    nc.gpsimd.partition_broadcast(bc[:], rr[:])
                    pend.append((pv, bc, h))

            for j in range(min(LOOK, n)):
                issue_score(j)
            for i in range(n):
                if i + LOOK < n:
                    issue_score(i + LOOK)
                yield
                issue_pv(i)
            while pend:
                flush_norm()

        # ---- o_proj: mc-pair granularity, [128,1024] out tiles ----
        wo_tiles = {}

        def load_wo(mc, eng=None):
            wos = wop.tile([128, N_FB * SCW], BF16, tag="wo", name="wo")
            (eng or nc.sync).dma_start(wos[:], wo_d[mc])
            wo_tiles[mc] = wos

        def oproj_row(sb, mcp, wr_eng=None, split_wr=False):
            """One [128,1024] output tile: s-block sb x mc pair (2mcp, 2mcp+1)."""
            sc, j = sb // 4, sb % 4
            ot = outp.tile([128, 1024], BF16, tag="out", name="out")
            for half in range(2):
                mc = 2 * mcp + half
                wos = wo_tiles[mc]
                op = ps_a.tile([128, SCW], F32, tag="acc", name="acc")
                for fb in range(N_FB):
                    nc.tensor.matmul(
                        op[:],
                        lhsT=AOT[(sc, fb)][:, j * 128:(j + 1) * 128],
                        rhs=wos[:, fb * SCW:(fb + 1) * SCW],
                        start=(fb == 0), stop=(fb == N_FB - 1),
                    )
                if half == 0:
                    nc.vector.tensor_copy(ot[:, 0:512], op[:])
                    if split_wr:  # drain the tail write in parallel halves
                        nc.scalar.dma_start(
                            out_d[sb * 128:(sb + 1) * 128,
                                  mcp * 1024:mcp * 1024 + 512], ot[:, 0:512])
                else:
                    nc.scalar.copy(ot[:, 512:1024], op[:])
            if split_wr:
                nc.sync.dma_start(
                    out_d[sb * 128:(sb + 1) * 128,
                          mcp * 1024 + 512:(mcp + 1) * 1024], ot[:, 512:1024])
            else:
                (wr_eng or nc.sync).dma_start(
                    out_d[sb * 128:(sb + 1) * 128, mcp * 1024:(mcp + 1) * 1024], ot[:])

        def oproj_c_stream():
            """Phase C: mcp-outer over s-rows 0..11. wo(0,1) preloaded in B2;
            pairs load just in time on gpsimd (sync carries the out writes);
            ring keeps the last 6 mc for phase D."""
            for mcp in range(N_MCP):
                if mcp + 1 < N_MCP:
                    load_wo(2 * mcp + 2, nc.gpsimd)
                    load_wo(2 * mcp + 3, nc.gpsimd)
                yield
                for sb in range(12):
                    oproj_row(sb, mcp)
                    yield

        def oproj_d():
            """Phase D: s-rows 12..15. mc 7..11 still in the wo ring (bufs=5);
            mc 0..5 reload staggered behind the resident mcps' rows. Writes
            alternate scalar/sync; wo pairs split gpsimd/sync."""
            def rows(mcp, last=False):
                for sb in range(12, 16):
                    oproj_row(sb, mcp, wr_eng=nc.scalar if sb % 2 else nc.sync,
                              split_wr=(last and sb == 15))
            rows(4)
            load_wo(6, nc.gpsimd)
            load_wo(7, nc.sync)
            rows(5)
            load_wo(0, nc.gpsimd)
            load_wo(1, nc.sync)
            rows(3)
            load_wo(2, nc.gpsimd)
            load_wo(3, nc.sync)
            rows(0)
            load_wo(4, nc.gpsimd)
            load_wo(5, nc.sync)
            rows(1)
            rows(2, last=True)

        def chain(*gens):
            for g in gens:
                yield from g

        def interleave(primary, filler, n_primary, n_filler, reserve=0,
                       ratio=None, drain=True):
            """Advance primary; between slots advance filler so both streams
            finish together (adaptive, or fixed `ratio`). Keep `reserve`
            filler units unexecuted; drain (or hand back) the remainder."""
            rem_p, rem_f = n_primary, n_filler
            acc = 0.0
            f_done = False
            for _ in primary:
                rem_p -= 1
                if not f_done:
                    acc += ratio if ratio is not None else rem_f / max(rem_p, 1)
                    while acc >= 1.0 and not f_done and rem_f > reserve:
                        try:
                            next(filler)
                            rem_f -= 1
                        except StopIteration:
                            f_done = True
                        acc -= 1.0
            if drain and not f_done:
                for _ in filler:
                    pass
            return filler if not f_done else None

        # ---- schedule ----
        carry = phase_a()

        def wo_preload():
            load_wo(0)
            load_wo(1)
            yield

        # B0..B2: attn(sc) ⋈ [carry + ht(sc+1) (+ wo preload) + qkv(sc+1)]
        for sc in range(3):
            primary = attn_stream(sc)
            parts = [carry] if carry is not None else []
            parts += [load_ht(sc + 1)]
            if sc == 2:  # stage wo(0..1) early for phase C
                parts += [wo_preload()]
            parts += [qkv_stream(sc + 1)]
            n_carry = 4 if sc == 0 else 0
            filler = chain(*parts)
            n_primary = NQ * (4 * sc + 4)
            n_filler = n_carry + 6 + 14 * N_OB + (1 if sc == 2 else 0)
            carry = interleave(primary, filler, n_primary, n_filler,
                               reserve=14)
        if carry is not None:
            for _ in carry:
                pass

        # C: attn(3) ⋈ o_proj rows 0..11 (adaptive ratio: 78 filler units
        # must last all 96 primary slots or the attn tail runs PE-starved)
        primary = attn_stream(3)
        filler = oproj_c_stream()
        interleave(primary, filler, NQ * 16, N_MCP * 13)

        # D: o_proj rows 12..15
        oproj_d()

    nc.compile()
    return nc


def prep_inputs(positions, hidden_states, w_qkv, w_o):
    """Host-side shard + relayout. Returns per-core input maps."""
    bf = ml_dtypes.bfloat16
    pos = np.asarray(positions).astype(np.float32)
    hidden = np.ascontiguousarray(np.asarray(hidden_states, dtype=np.float32))
    w_qkv = np.asarray(w_qkv, dtype=np.float32)
    w_o = np.asarray(w_o, dtype=np.float32)

    # rope tables (neox): freqs [S, 64]
    inv_freq = 1.0 / (ROPE_THETA ** (np.arange(0, D, 2, dtype=np.float32) / D))
    freqs = pos[:, None] * inv_freq[None, :]
    cos = np.cos(freqs).T.astype(np.float32)   # [64, S]
    sin = np.sin(freqs).T.astype(np.float32)
    cosf = np.concatenate([cos, cos], axis=0).astype(bf)    # [128, S]
    sinf = np.concatenate([-sin, sin], axis=0).astype(bf)

    triu = np.triu(np.ones((128, 128), np.float32)).astype(bf)  # [k, q]: q >= k
    ones_col = np.ones((128, 1), np.float32).astype(bf)
    ident = np.eye(128, dtype=np.float32).astype(bf)

    # ht[sc, hbp, p, j*512+c] = hidden[sc*512+c, (2*hbp+j)*128+p]
    ht = np.ascontiguousarray(
        hidden.reshape(N_SC, SCW, N_HBP, 2, 128).transpose(0, 2, 4, 3, 1)
    ).reshape(N_SC, N_HBP, 128, 1024).astype(bf)

    in_maps = []
    for c in range(N_CORES):
        q_rows = w_qkv[c * NQ * D:(c + 1) * NQ * D]          # [768, 6144]
        k_rows = w_qkv[HID + c * D:HID + (c + 1) * D]        # [128, 6144]
        v_rows = w_qkv[HID + 8 * D + c * D:HID + 8 * D + (c + 1) * D]
        wq_c = np.concatenate([q_rows, k_rows, v_rows], axis=0)  # [1024, 6144]
        # wq[ob, p, hb, o] = wq_c[ob*128+o, hb*128+p]
        wq_arr = np.ascontiguousarray(
            wq_c.reshape(N_OB, 128, N_HB, 128).transpose(0, 3, 2, 1)).astype(bf)
        # wqa[g, pr, p, (i*4+m)*128+o] = wq_c[pair[pr][i]*128+o, (4g+m)*128+p]
        wq_blk = wq_c.reshape(N_OB, 128, 12, 4, 128)  # [ob, o, g, m, p]
        wqa_arr = np.empty((12, 4, 128, 1024), np.float32)
        for pr, obs in enumerate(((6, 7), (0, 1), (2, 3), (4, 5))):
            blk = wq_blk[list(obs)]                   # [i, o, g, m, p]
            wqa_arr[:, pr] = blk.transpose(2, 4, 0, 3, 1).reshape(12, 128, 1024)
        wqa_arr = np.ascontiguousarray(wqa_arr).astype(bf)
        wo_c = (w_o[:, c * NQ * D:(c + 1) * NQ * D] * ATTN_MULT).T  # [768, 6144]
        # wo[mc, p, fb, m] = wo_c[fb*128+p, mc*512+m]
        wo_arr = np.ascontiguousarray(
            wo_c.reshape(N_FB, 128, N_MC, SCW).transpose(2, 1, 0, 3)).astype(bf)
        in_maps.append({
            "ht": ht, "wq": wq_arr, "wqa": wqa_arr, "wo": wo_arr,
            "cosf": cosf, "sinf": sinf, "triu": triu,
            "ones_col": ones_col, "ident": ident,
            "negcap": np.full((128, 1), -SOFTCAP, np.float32),
        })
    return in_maps


_NC_CACHE = None


def _get_nc():
    global _NC_CACHE
    if _NC_CACHE is None:
        _NC_CACHE = build_nc()
    return _NC_CACHE


def kernel(positions, hidden_states, w_qkv, w_o, _trace=False, _trace_kwargs=None):
    nc = _get_nc()
    in_maps = prep_inputs(positions, hidden_states, w_qkv, w_o)
    res = run_bass_kernel_spmd(nc, in_maps, list(range(N_CORES)),
                               trace=_trace, **(_trace_kwargs or {}))
    out = np.zeros((S, HID), np.float32)
    for c in range(N_CORES):
        out += np.asarray(res.results[c]["out"]).astype(np.float32)
    out = out.astype(np.asarray(hidden_states).dtype)
    kernel.last_results = res
    return out
